# revision 1
# baseline (speedup 1.0000x reference)
"""GAT (3-layer, N=50000, E=1.6M, D=128) on 8 Trainium2 NeuronCores.

Strategy (dst-sharded ELL):
  - Nodes sharded by destination across 8 cores (6250 dst/core).
  - Per core, dsts are sorted by (in-degree from low table half, then high
    half) desc and grouped into 49 groups of 128.  Edges live in a padded
    ELL layout [128 dst, K slots] per group; the slots are split into a
    "lo" block (source rows < HALF) and a "hi" block so the int16 indices
    of dma_gather can address a 25088-row table half each.
  - Per layer each core computes h = z @ W for its shard (feature-major
    via PE), builds 512B gather rows [h fp16 x128 | asrc fp32 | junk],
    and an AllGather replicates the full table.
  - Edge phase per group: two dma_gathers fetch all slot rows; softmax
    (leaky-relu, per-dst max, exp+accum, reciprocal) is native
    per-partition work; aggregation is an in-place DVE multiply plus a
    reduce over slots; PE transposes move results to feature-major.
  - BatchNorm: free-axis reductions + a [128,2] AllReduce; normalize+ReLU
    is one ACT op.

kernel() accepts FULL inputs and returns the FULL [50000,128] output.
"""

import numpy as np

import concourse.bacc as bacc
import concourse.mybir as mybir
import concourse.tile as tile
from concourse.bass_utils import run_bass_kernel_spmd

F32 = mybir.dt.float32
F16 = mybir.dt.float16
I16 = mybir.dt.int16
AX = mybir.AxisListType
OP = mybir.AluOpType
AF = mybir.ActivationFunctionType

NCORES = 8
D = 128
L = 3
EPS = 1e-5
SLOPE = 0.2
NEG_BIG = -1e30
ROWE = 256          # fp16 elems per table row (512B): 128 h + 2 asrc + junk
ASRC_F32_COL = 64   # fp32-view column of asrc within a row


# ----------------------------------------------------------------- host prep
def _build_host(x, edge_index, W, a_src, a_dst):
    N = x.shape[0]
    NLOC = N // NCORES
    G = NLOC // 128 + 1          # always >= 1 junk row per core block
    NPAD = G * 128
    HALF = (NCORES // 2) * NPAD

    src = np.concatenate([edge_index[0], np.arange(N)]).astype(np.int64)
    dst = np.concatenate([edge_index[1], np.arange(N)]).astype(np.int64)

    # global node id -> table row needs perms first; two passes.
    # pass 1: per-core degree data and perm
    pc = []
    for c in range(NCORES):
        m = (dst >= c * NLOC) & (dst < (c + 1) * NLOC)
        s_c = src[m]
        d_c = dst[m] - c * NLOC
        deg_c = np.bincount(d_c, minlength=NLOC)
        srclo = s_c < (NCORES // 2) * NLOC   # owner core < NCORES/2
        nlo_c = np.bincount(d_c[srclo], minlength=NLOC)
        nhi_c = deg_c - nlo_c
        perm_c = np.lexsort((-nhi_c, -nlo_c))
        order = np.argsort(d_c, kind="stable")
        pc.append(dict(s=s_c[order], d=d_c[order], deg=deg_c, nlo=nlo_c,
                       nhi=nhi_c, perm=perm_c))

    tablerow = np.empty(N, np.int64)
    for c in range(NCORES):
        inv = np.empty(NLOC, np.int64)
        inv[pc[c]["perm"]] = np.arange(NLOC)
        tablerow[c * NLOC:(c + 1) * NLOC] = c * NPAD + inv

    Klo = np.zeros(G, np.int64)
    Khi = np.zeros(G, np.int64)
    for g in range(G):
        for c in range(NCORES):
            rows = pc[c]["perm"][g * 128:(g + 1) * 128]
            if len(rows):
                Klo[g] = max(Klo[g], pc[c]["nlo"][rows].max())
                Khi[g] = max(Khi[g], pc[c]["nhi"][rows].max())
    Klo = np.maximum(Klo, 1)
    Khi = np.maximum(Khi, 1)
    Kt = Klo + Khi
    offs = np.zeros(G + 1, np.int64)
    np.cumsum(Kt, out=offs[1:])
    TOTK = int(offs[-1])

    def pack16(stream):  # stream [n] int64 -> [128, n//16] int16 wrapped
        n = len(stream)
        arr = stream.reshape(n // 16, 16).T.astype(np.int16)  # [16, n/16]
        return np.tile(arr, (8, 1))

    idx_maps, mask_maps, xt_maps = [], [], []
    for c in range(NCORES):
        P = pc[c]
        starts = np.zeros(NLOC + 1, np.int64)
        np.cumsum(P["deg"], out=starts[1:])
        idx16 = np.zeros((128, 8 * TOTK), np.int16)
        JUNK = NLOC  # first junk row in each half (asrc = -1e30 on device)
        for g in range(G):
            rows = P["perm"][g * 128:(g + 1) * 128]
            kl, kh = int(Klo[g]), int(Khi[g])
            lo_st = np.full((kl, 128), JUNK, np.int64)   # slot-major [k, p]
            hi_st = np.full((kh, 128), JUNK, np.int64)
            for p, dloc in enumerate(rows):
                es = P["s"][starts[dloc]:starts[dloc] + P["deg"][dloc]]
                rs = tablerow[es]
                rlo = rs[rs < HALF]
                rhi = rs[rs >= HALF] - HALF
                lo_st[:len(rlo), p] = rlo
                hi_st[:len(rhi), p] = rhi
            o16 = 8 * offs[g]
            idx16[:, o16:o16 + 8 * kl] = pack16(lo_st.ravel())
            idx16[:, o16 + 8 * kl:o16 + 8 * (kl + kh)] = pack16(hi_st.ravel())
        idx_maps.append(idx16)
        xt_maps.append(np.ascontiguousarray(
            x[c * NLOC + P["perm"]].T.astype(np.float32)))

    Wa = np.stack(
        [np.stack([W[l] @ a_src[l], W[l] @ a_dst[l]], axis=-1) for l in range(L)]
    ).astype(np.float32)  # [L,128,2]

    return dict(N=N, NLOC=NLOC, G=G, NPAD=NPAD, HALF=HALF,
                Klo=[int(k) for k in Klo], Khi=[int(k) for k in Khi],
                offs=[int(o) for o in offs], TOTK=TOTK,
                perms=[p["perm"] for p in pc],
                idx_maps=idx_maps, xt_maps=xt_maps, Wa=Wa)


# ------------------------------------------------------------- device program
def _build_program(NLOC, G, NPAD, HALF, Klo, Khi, offs, TOTK, debug=False):
    TROWS = NCORES * NPAD
    nc = bacc.Bacc("TRN2", num_devices=NCORES)
    dbg = {}
    if debug:
        K0 = Klo[0] + Khi[0]
        dbg["hT"] = nc.dram_tensor("d_hT", [128, NLOC], F32, kind="ExternalOutput")
        dbg["table"] = nc.dram_tensor("d_table", [TROWS, ROWE], F16,
                                      kind="ExternalOutput")
        dbg["gt0"] = nc.dram_tensor("d_gt0", [128, K0, ROWE], F16,
                                    kind="ExternalOutput")
        dbg["u0"] = nc.dram_tensor("d_u0", [128, K0], F32, kind="ExternalOutput")
        dbg["s0"] = nc.dram_tensor("d_s0", [128, 1], F32, kind="ExternalOutput")
        dbg["zt0"] = nc.dram_tensor("d_zt0", [128, 128], F32, kind="ExternalOutput")
        dbg["zagg"] = nc.dram_tensor("d_zagg", [128, NPAD], F32,
                                     kind="ExternalOutput")

    x_in = nc.dram_tensor("xt", [128, NLOC], F32, kind="ExternalInput")
    w_in = nc.dram_tensor("w", [L, 128, 128], F32, kind="ExternalInput")
    wa_in = nc.dram_tensor("wa", [L, 128, 2], F32, kind="ExternalInput")
    idx_in = nc.dram_tensor("idx", [128, 8 * TOTK], I16, kind="ExternalInput")
    id_in = nc.dram_tensor("ident", [128, 128], F32, kind="ExternalInput")
    out_t = nc.dram_tensor("zout", [128, NLOC], F32, kind="ExternalOutput")

    NCHUNK = (NLOC + 511) // 512
    rg = [[i for i in range(NCORES)]]

    import os as _os2
    with tile.TileContext(nc, linearize=_os2.environ.get("KLIN") == "1") as tc:
        from contextlib import ExitStack
        with ExitStack() as ctx:
            const = ctx.enter_context(tc.tile_pool(name="const", bufs=1))
            npool = ctx.enter_context(tc.tile_pool(name="npool", bufs=2))
            hpool = ctx.enter_context(tc.tile_pool(name="hpool", bufs=1))
            apool = ctx.enter_context(tc.tile_pool(name="apool", bufs=2))
            zgpool = ctx.enter_context(tc.tile_pool(name="zgpool", bufs=1))
            spool = ctx.enter_context(tc.tile_pool(name="spool", bufs=3))
            gpool = ctx.enter_context(tc.tile_pool(name="gpool", bufs=2))
            ipool = ctx.enter_context(tc.tile_pool(name="ipool", bufs=2))
            zpool = ctx.enter_context(tc.tile_pool(name="zpool", bufs=2))
            pp = ctx.enter_context(tc.tile_pool(name="pp", bufs=2, space="PSUM"))
            ppt = ctx.enter_context(tc.tile_pool(name="ppt", bufs=2, space="PSUM"))
            dpool = ctx.enter_context(tc.tile_pool(name="dpool", bufs=2, space="DRAM"))
            dtab = ctx.enter_context(tc.tile_pool(name="dtab", bufs=2, space="DRAM"))

            ident = const.tile([128, 128], F32)
            nc.sync.dma_start(ident[:], id_in[:, :])
            zeros1 = const.tile([128, 1], F32)
            nc.vector.memset(zeros1[:], 0.0)
            negbig = const.tile([2, 128], F32)
            nc.vector.memset(negbig[:], NEG_BIG)
            w_sb = const.tile([128, L * 128], F32)
            wa_sb = const.tile([128, L * 2], F32)
            for l in range(L):
                nc.sync.dma_start(w_sb[:, l * 128:(l + 1) * 128], w_in[l, :, :])
                nc.sync.dma_start(wa_sb[:, l * 2:(l + 1) * 2], wa_in[l, :, :])

            znT = npool.tile([128, NLOC], F32, tag="znT")
            nc.sync.dma_start(znT[:], x_in[:, :])

            for l in range(L):
                # ---------------- node phase: h, asrc/adst, table build ----
                hT = hpool.tile([128, NPAD], F32, tag="hT")
                if NPAD > NLOC:
                    nc.vector.memset(hT[:, NLOC:NPAD], 0.0)
                avb = dpool.tile([2, NPAD], F32, tag="avb")
                nc.sync.dma_start(avb[:2, NLOC:NPAD], negbig[:2, :NPAD - NLOC])
                for j in range(NCHUNK):
                    a, bnd = j * 512, min((j + 1) * 512, NLOC)
                    w_ = bnd - a
                    ph = pp.tile([128, 512], F32, tag="ph")
                    nc.tensor.matmul(ph[:, :w_], w_sb[:, l * 128:(l + 1) * 128],
                                     znT[:, a:bnd], start=True, stop=True)
                    nc.vector.tensor_copy(hT[:, a:bnd], ph[:, :w_])
                    pa = pp.tile([2, 512], F32, tag="pa")
                    nc.tensor.matmul(pa[:2, :w_], wa_sb[:, l * 2:(l + 1) * 2],
                                     znT[:, a:bnd], start=True, stop=True)
                    avc = apool.tile([2, 512], F32, tag="avc")
                    nc.vector.tensor_copy(avc[:2, :w_], pa[:2, :w_])
                    nc.sync.dma_start(avb[:2, a:bnd], avc[:2, :w_])
                asrc_g = npool.tile([128, G], F32, tag="asrc_g")
                adst_g = npool.tile([128, G], F32, tag="adst_g")
                nc.sync.dma_start(
                    asrc_g[:], avb[0, :].rearrange("(g p) -> p g", p=128))
                nc.sync.dma_start(
                    adst_g[:], avb[1, :].rearrange("(g p) -> p g", p=128))

                # table rows: transpose h per group, cast fp16, add asrc col
                rowbuf = npool.tile([128, G, 132], F16, tag="rowbuf")
                nc.vector.memset(rowbuf[:, :, 130:132], 0.0)
                for g in range(G):
                    pt = ppt.tile([128, 128], F32, tag="pt")
                    nc.tensor.matmul(pt[:], hT[:, g * 128:(g + 1) * 128],
                                     ident[:], is_transpose=True,
                                     start=True, stop=True)
                    nc.vector.tensor_copy(rowbuf[:, g, 0:128], pt[:])
                rb32 = rowbuf[:].bitcast(F32)  # [128, G, 66]
                nc.vector.tensor_copy(rb32[:, :, 64:65], asrc_g[:].unsqueeze(-1))

                stag = dpool.tile([NPAD, ROWE], F16, tag="stag")
                nc.sync.dma_start(
                    stag[:, 0:132].rearrange("(g p) e -> p g e", p=128),
                    rowbuf[:])
                table = dtab.tile([TROWS, ROWE], F16, tag="table")
                nc.gpsimd.collective_compute(
                    "AllGather", OP.bypass, replica_groups=rg,
                    ins=[stag[:, :]], outs=[table[:, :]])
                if debug and l == 0:
                    nc.sync.dma_start(dbg["hT"][:, :], hT[:])
                    tbs = npool.tile([128, G * NCORES, 132], F16, tag="tbs")
                    nc.sync.dma_start(
                        tbs[:], table[:, 0:132].rearrange("(g p) e -> p g e",
                                                          p=128))
                    nc.sync.dma_start(
                        dbg["table"][:, 0:132].rearrange("(g p) e -> p g e",
                                                         p=128), tbs[:])

                # ---------------- edge phase ------------------------------
                zaggT = zgpool.tile([128, NPAD], F32, tag="zaggT")
                for g in range(G):
                    kl, kh = Klo[g], Khi[g]
                    K = kl + kh
                    o = offs[g]
                    idxt = ipool.tile([128, 8 * K], I16, tag="idxt")
                    nc.sync.dma_start(idxt[:],
                                      idx_in[:, 8 * o:8 * (o + K)])
                    gt = gpool.tile([128, K, ROWE], F16, tag="gt")
                    if _os2.environ.get("KBISECT") == "2":
                        nc.vector.memset(gt[:], 0.0)
                    # firmware ring limit: keep gathers at <=1024 indices
                    SMAX = 8
                    for (base, cnt, toff) in ([] if _os2.environ.get("KBISECT") == "2" else [(0, kl, 0), (kl, kh, 0)]):
                        tb = table[0:HALF, :] if base == 0 else \
                            table[HALF:TROWS, :]
                        for s0 in range(0, cnt, SMAX):
                            s1 = min(s0 + SMAX, cnt)
                            nc.gpsimd.dma_gather(
                                gt[:, base + s0:base + s1, :], tb,
                                idxt[:, 8 * (base + s0):8 * (base + s1)],
                                128 * (s1 - s0), 128 * (s1 - s0), ROWE)

                    import os
                    if os.environ.get("KBISECT") == "1":
                        zt = zpool.tile([128, 128], F32, tag="zt")
                        nc.vector.tensor_copy(zt[:], gt[:, 0, 0:128])
                        pz = ppt.tile([128, 128], F32, tag="pt")
                        nc.tensor.matmul(pz[:], zt[:], ident[:],
                                         is_transpose=True, start=True,
                                         stop=True)
                        nc.vector.tensor_copy(
                            zaggT[:, g * 128:(g + 1) * 128], pz[:])
                        continue
                    gt32 = gt[:].bitcast(F32)  # [128, K, 128]
                    u = spool.tile([128, K], F32, tag="u")
                    nc.vector.tensor_scalar(
                        u[:], gt32[:, :, ASRC_F32_COL:ASRC_F32_COL + 1].squeeze(-1),
                        adst_g[:, g:g + 1], None, op0=OP.add)
                    u2 = spool.tile([128, K], F32, tag="u2")
                    nc.vector.tensor_scalar_mul(u2[:], u[:], SLOPE)
                    e = spool.tile([128, K], F32, tag="e")
                    nc.vector.tensor_tensor(e[:], u[:], u2[:], OP.max)
                    mneg = spool.tile([128, 1], F32, tag="mneg")
                    nc.vector.tensor_reduce(mneg[:], e[:], axis=AX.X, op=OP.max,
                                            negate=True)
                    p16 = spool.tile([128, K], F16, tag="p16")
                    s = spool.tile([128, 1], F32, tag="s")
                    nc.scalar.activation(p16[:], e[:], AF.Exp,
                                         bias=mneg[:, 0:1], scale=1.0,
                                         accum_out=s[:, 0:1])
                    rs = spool.tile([128, 1], F32, tag="rs")
                    nc.vector.reciprocal(rs[:], s[:])
                    pn = spool.tile([128, K], F16, tag="pn")
                    nc.vector.tensor_scalar(pn[:], p16[:], rs[:, 0:1], None,
                                            op0=OP.mult)

                    nc.vector.tensor_tensor(
                        gt[:, :, 0:128], gt[:, :, 0:128],
                        pn[:].unsqueeze(-1).broadcast_to((128, K, 128)), OP.mult)
                    zt = zpool.tile([128, 128], F32, tag="zt")
                    nc.vector.tensor_reduce(
                        zt[:], gt[:, :, 0:128].rearrange("p k f -> p f k"),
                        axis=AX.X, op=OP.add)
                    pz = ppt.tile([128, 128], F32, tag="pt")
                    nc.tensor.matmul(pz[:], zt[:], ident[:], is_transpose=True,
                                     start=True, stop=True)
                    nc.vector.tensor_copy(zaggT[:, g * 128:(g + 1) * 128], pz[:])
                    if debug and l == 0 and g == 0:
                        nc.sync.dma_start(dbg["gt0"][:, :, :], gt[:])
                        nc.sync.dma_start(dbg["u0"][:, :], u[:])
                        nc.sync.dma_start(dbg["s0"][:, :], s[:])
                        nc.sync.dma_start(dbg["zt0"][:, :], zt[:])

                # ---------------- BN + ReLU -------------------------------
                if debug and l == 0:
                    nc.sync.dma_start(dbg["zagg"][:, :], zaggT[:])
                stats = npool.tile([128, 2], F32, tag="stats")
                nc.vector.tensor_reduce(stats[:, 0:1], zaggT[:, :NLOC],
                                        axis=AX.X, op=OP.add)
                sqp = npool.tile([128, NCHUNK], F32, tag="sqp")
                for j in range(NCHUNK):
                    a, bnd = j * 512, min((j + 1) * 512, NLOC)
                    w_ = bnd - a
                    scr = pp.tile([128, 512], F32, tag="ph")
                    nc.vector.scalar_tensor_tensor(
                        scr[:, :w_], zaggT[:, a:bnd], 0.0, zaggT[:, a:bnd],
                        op0=OP.add, op1=OP.mult,
                        accum_out=sqp[:, j:j + 1])
                nc.vector.tensor_reduce(stats[:, 1:2], sqp[:], axis=AX.X,
                                        op=OP.add)

                stb = dpool.tile([128, 2], F32, tag="stb")
                nc.sync.dma_start(stb[:, :], stats[:])
                nc.gpsimd.collective_compute(
                    "AllReduce", OP.add, replica_groups=rg,
                    ins=[stb[:, :]], outs=[stb[:, :]])
                gstats = npool.tile([128, 2], F32, tag="gstats")
                nc.sync.dma_start(gstats[:], stb[:, :])

                mu = npool.tile([128, 1], F32, tag="mu")
                nc.vector.tensor_scalar_mul(mu[:], gstats[:, 0:1],
                                            1.0 / (NLOC * NCORES))
                msq = npool.tile([128, 1], F32, tag="msq")
                nc.vector.tensor_scalar_mul(msq[:], gstats[:, 1:2],
                                            1.0 / (NLOC * NCORES))
                mu2 = npool.tile([128, 1], F32, tag="mu2")
                nc.vector.tensor_tensor(mu2[:], mu[:], mu[:], OP.mult)
                var = npool.tile([128, 1], F32, tag="var")
                nc.vector.scalar_tensor_tensor(var[:], msq[:], EPS, mu2[:],
                                               op0=OP.add, op1=OP.subtract)
                sd = npool.tile([128, 1], F32, tag="sd")
                nc.scalar.activation(sd[:], var[:], AF.Sqrt,
                                     bias=zeros1[:, 0:1], scale=1.0)
                rstd = npool.tile([128, 1], F32, tag="rstd")
                nc.vector.reciprocal(rstd[:], sd[:])
                nmr = npool.tile([128, 1], F32, tag="nmr")
                nc.vector.scalar_tensor_tensor(nmr[:], mu[:], -1.0, rstd[:],
                                               op0=OP.mult, op1=OP.mult)
                zn2 = npool.tile([128, NLOC], F32, tag="znT")
                nc.scalar.activation(zn2[:], zaggT[:, :NLOC], AF.Relu,
                                     bias=nmr[:, 0:1], scale=rstd[:, 0:1])
                znT = zn2

            nc.sync.dma_start(out_t[:, :], znT[:])

    nc.compile()
    return nc


_CACHE = {}


def _get_program(key, *args, **kw):
    if key not in _CACHE:
        _CACHE[key] = _build_program(*args, **kw)
    return _CACHE[key]


def kernel(x, edge_index, W, a_src, a_dst, b):
    x = np.asarray(x, np.float32)
    edge_index = np.asarray(edge_index)
    W = np.asarray(W, np.float32)
    a_src = np.asarray(a_src, np.float32)
    a_dst = np.asarray(a_dst, np.float32)

    hp = _build_host(x, edge_index, W, a_src, a_dst)
    NLOC, G, NPAD, TOTK = hp["NLOC"], hp["G"], hp["NPAD"], hp["TOTK"]
    key = (NLOC, G, tuple(hp["Klo"]), tuple(hp["Khi"]))
    nc = _get_program(key, NLOC, G, NPAD, hp["HALF"], hp["Klo"], hp["Khi"],
                      hp["offs"], TOTK)

    ident = np.eye(128, dtype=np.float32)
    in_maps = []
    for c in range(NCORES):
        in_maps.append({
            "xt": hp["xt_maps"][c],
            "w": W,
            "wa": hp["Wa"],
            "idx": hp["idx_maps"][c],
            "ident": ident,
        })

    res = run_bass_kernel_spmd(nc, in_maps, core_ids=list(range(NCORES)))

    N = x.shape[0]
    out = np.empty((N, 128), np.float32)
    for c in range(NCORES):
        zc = res.results[c]["zout"]  # [128, NLOC]
        out[c * NLOC + hp["perms"][c]] = zc.T
    return out


def profile_exec_ns(inputs):
    """Run once with tracing and return HW exec time in ns (or None)."""
    x = np.asarray(inputs["x"], np.float32)
    hp = _build_host(x, np.asarray(inputs["edge_index"]),
                     np.asarray(inputs["W"], np.float32),
                     np.asarray(inputs["a_src"], np.float32),
                     np.asarray(inputs["a_dst"], np.float32))
    key = (hp["NLOC"], hp["G"], tuple(hp["Klo"]), tuple(hp["Khi"]))
    nc = _get_program(key, hp["NLOC"], hp["G"], hp["NPAD"], hp["HALF"],
                      hp["Klo"], hp["Khi"], hp["offs"], hp["TOTK"])
    ident = np.eye(128, dtype=np.float32)
    in_maps = [{"xt": hp["xt_maps"][c], "w": np.asarray(inputs["W"], np.float32),
                "wa": hp["Wa"], "idx": hp["idx_maps"][c], "ident": ident}
               for c in range(NCORES)]
    try:
        res = run_bass_kernel_spmd(nc, in_maps, core_ids=list(range(NCORES)),
                                   trace=True)
        return res.exec_time_ns
    except Exception as ex:
        print("profile failed:", ex)
        return None



# revision 11
# speedup vs baseline: 545.8458x; 545.8458x over previous
"""GAT (3-layer, N=50000, E=1.6M, D=128) on 8 Trainium2 NeuronCores.

Strategy (dst-sharded ELL):
  - Nodes sharded by destination across 8 cores (6250 dst/core).
  - Per core, dsts are sorted by (in-degree from low table half, then high
    half) desc and grouped into 49 groups of 128.  Edges live in a padded
    ELL layout [128 dst, K slots] per group; the slots are split into a
    "lo" block (source rows < HALF) and a "hi" block so the int16 indices
    of dma_gather can address a 25088-row table half each.
  - Per layer each core computes h = z @ W for its shard (feature-major
    via PE), builds 512B gather rows [h fp16 x128 | asrc fp32 | junk],
    and an AllGather replicates the full table.
  - Edge phase per group: two dma_gathers fetch all slot rows; softmax
    (leaky-relu, per-dst max, exp+accum, reciprocal) is native
    per-partition work; aggregation is an in-place DVE multiply plus a
    reduce over slots; PE transposes move results to feature-major.
  - BatchNorm: free-axis reductions + a [128,2] AllReduce; normalize+ReLU
    is one ACT op.

kernel() accepts FULL inputs and returns the FULL [50000,128] output.
"""

import numpy as np

import concourse.bacc as bacc
import concourse.mybir as mybir
import concourse.tile as tile
from concourse.bass_utils import run_bass_kernel_spmd

F32 = mybir.dt.float32
F16 = mybir.dt.float16
I16 = mybir.dt.int16
AX = mybir.AxisListType
OP = mybir.AluOpType
AF = mybir.ActivationFunctionType

NCORES = 8
D = 128
L = 3
EPS = 1e-5
SLOPE = 0.2
NEG_BIG = -1e30
ROWE = 256          # fp16 elems per table row (512B): 128 h + 2 asrc + junk
ASRC_F32_COL = 64   # fp32-view column of asrc within a row


# ----------------------------------------------------------------- host prep
def _build_host(x, edge_index, W, a_src, a_dst):
    N = x.shape[0]
    NLOC = N // NCORES
    G = NLOC // 128 + 1          # always >= 1 junk row per core block
    NPAD = G * 128
    HALF = (NCORES // 2) * NPAD

    src = np.concatenate([edge_index[0], np.arange(N)]).astype(np.int64)
    dst = np.concatenate([edge_index[1], np.arange(N)]).astype(np.int64)

    # global node id -> table row needs perms first; two passes.
    # pass 1: per-core degree data and perm
    pc = []
    for c in range(NCORES):
        m = (dst >= c * NLOC) & (dst < (c + 1) * NLOC)
        s_c = src[m]
        d_c = dst[m] - c * NLOC
        deg_c = np.bincount(d_c, minlength=NLOC)
        srclo = s_c < (NCORES // 2) * NLOC   # owner core < NCORES/2
        nlo_c = np.bincount(d_c[srclo], minlength=NLOC)
        nhi_c = deg_c - nlo_c
        perm_c = np.lexsort((-nhi_c, -nlo_c))
        order = np.argsort(d_c, kind="stable")
        pc.append(dict(s=s_c[order], d=d_c[order], deg=deg_c, nlo=nlo_c,
                       nhi=nhi_c, perm=perm_c))

    tablerow = np.empty(N, np.int64)
    for c in range(NCORES):
        inv = np.empty(NLOC, np.int64)
        inv[pc[c]["perm"]] = np.arange(NLOC)
        tablerow[c * NLOC:(c + 1) * NLOC] = c * NPAD + inv

    Klo = np.zeros(G, np.int64)
    Khi = np.zeros(G, np.int64)
    for g in range(G):
        for c in range(NCORES):
            rows = pc[c]["perm"][g * 128:(g + 1) * 128]
            if len(rows):
                Klo[g] = max(Klo[g], pc[c]["nlo"][rows].max())
                Khi[g] = max(Khi[g], pc[c]["nhi"][rows].max())
    Klo = np.maximum(Klo, 1)
    Khi = np.maximum(Khi, 1)
    Kt = Klo + Khi
    offs = np.zeros(G + 1, np.int64)
    np.cumsum(Kt, out=offs[1:])
    TOTK = int(offs[-1])

    def pack16(stream):  # stream [n] int64 -> [128, n//16] int16 wrapped
        n = len(stream)
        arr = stream.reshape(n // 16, 16).T.astype(np.int16)  # [16, n/16]
        return np.tile(arr, (8, 1))

    idx_maps, mask_maps, xt_maps = [], [], []
    for c in range(NCORES):
        P = pc[c]
        starts = np.zeros(NLOC + 1, np.int64)
        np.cumsum(P["deg"], out=starts[1:])
        idx16 = np.zeros((128, 8 * TOTK), np.int16)
        JUNK = NLOC  # first junk row in each half (asrc = -1e30 on device)
        for g in range(G):
            rows = P["perm"][g * 128:(g + 1) * 128]
            kl, kh = int(Klo[g]), int(Khi[g])
            lo_st = np.full((kl, 128), JUNK, np.int64)   # slot-major [k, p]
            hi_st = np.full((kh, 128), JUNK, np.int64)
            for p, dloc in enumerate(rows):
                es = P["s"][starts[dloc]:starts[dloc] + P["deg"][dloc]]
                rs = tablerow[es]
                rlo = rs[rs < HALF]
                rhi = rs[rs >= HALF] - HALF
                lo_st[:len(rlo), p] = rlo
                hi_st[:len(rhi), p] = rhi
            o16 = 8 * offs[g]
            idx16[:, o16:o16 + 8 * kl] = pack16(lo_st.ravel())
            idx16[:, o16 + 8 * kl:o16 + 8 * (kl + kh)] = pack16(hi_st.ravel())
        idx_maps.append(idx16)
        xt_maps.append(np.ascontiguousarray(
            x[c * NLOC + P["perm"]].T.astype(np.float16)))

    Wa = np.stack(
        [np.stack([W[l] @ a_src[l], W[l] @ a_dst[l]], axis=-1) for l in range(L)]
    ).astype(np.float16)  # [L,128,2]

    return dict(N=N, NLOC=NLOC, G=G, NPAD=NPAD, HALF=HALF,
                Klo=[int(k) for k in Klo], Khi=[int(k) for k in Khi],
                offs=[int(o) for o in offs], TOTK=TOTK,
                perms=[p["perm"] for p in pc],
                idx_maps=idx_maps, xt_maps=xt_maps, Wa=Wa)


# ------------------------------------------------------------- device program
def _build_program(NLOC, G, NPAD, HALF, Klo, Khi, offs, TOTK, debug=False):
    TROWS = NCORES * NPAD
    nc = bacc.Bacc("TRN2", num_devices=NCORES, num_swdge_queues=4)
    dbg = {}
    if debug:
        K0 = Klo[0] + Khi[0]
        dbg["hT"] = nc.dram_tensor("d_hT", [128, NLOC], F32, kind="ExternalOutput")
        dbg["table"] = nc.dram_tensor("d_table", [TROWS, ROWE], F16,
                                      kind="ExternalOutput")
        dbg["gt0"] = nc.dram_tensor("d_gt0", [128, K0, ROWE], F16,
                                    kind="ExternalOutput")
        dbg["u0"] = nc.dram_tensor("d_u0", [128, K0], F32, kind="ExternalOutput")
        dbg["s0"] = nc.dram_tensor("d_s0", [128, 1], F32, kind="ExternalOutput")
        dbg["zt0"] = nc.dram_tensor("d_zt0", [128, 128], F32, kind="ExternalOutput")
        dbg["zagg"] = nc.dram_tensor("d_zagg", [128, NPAD], F32,
                                     kind="ExternalOutput")

    x_in = nc.dram_tensor("xt", [128, NLOC], F16, kind="ExternalInput")
    w_in = nc.dram_tensor("w", [L, 128, 128], F16, kind="ExternalInput")
    wa_in = nc.dram_tensor("wa", [L, 128, 2], F16, kind="ExternalInput")
    idx_in = nc.dram_tensor("idx", [128, 8 * TOTK], I16, kind="ExternalInput")
    id_in = nc.dram_tensor("ident", [128, 128], F16, kind="ExternalInput")
    out_t = nc.dram_tensor("zout", [128, NLOC], F32, kind="ExternalOutput")

    NCHUNK = (NLOC + 511) // 512
    rg = [[i for i in range(NCORES)]]

    import os as _os2
    with tile.TileContext(nc, linearize=_os2.environ.get("KLIN") == "1") as tc:
        from contextlib import ExitStack
        with ExitStack() as ctx:
            const = ctx.enter_context(tc.tile_pool(name="const", bufs=1))
            npool = ctx.enter_context(tc.tile_pool(name="npool", bufs=2))
            hpool = ctx.enter_context(tc.tile_pool(name="hpool", bufs=1))
            apool = ctx.enter_context(tc.tile_pool(name="apool", bufs=2))
            zgpool = ctx.enter_context(tc.tile_pool(name="zgpool", bufs=1))
            spool = ctx.enter_context(tc.tile_pool(name="spool", bufs=3))
            gpool = ctx.enter_context(tc.tile_pool(name="gpool", bufs=2))
            ipool = ctx.enter_context(tc.tile_pool(name="ipool", bufs=4))
            zpool = ctx.enter_context(tc.tile_pool(name="zpool", bufs=2))
            pp = ctx.enter_context(tc.tile_pool(name="pp", bufs=2, space="PSUM"))
            ppt = ctx.enter_context(tc.tile_pool(name="ppt", bufs=2, space="PSUM"))
            dpool = ctx.enter_context(tc.tile_pool(name="dpool", bufs=2, space="DRAM"))
            dtab = ctx.enter_context(tc.tile_pool(name="dtab", bufs=2, space="DRAM"))

            ident = const.tile([128, 128], F16)
            nc.sync.dma_start(ident[:], id_in[:, :])
            zeros1 = const.tile([128, 1], F32)
            nc.vector.memset(zeros1[:], 0.0)
            negbig = const.tile([2, 128], F32)
            nc.vector.memset(negbig[:], NEG_BIG)
            w_sb = const.tile([128, L * 128], F16)
            wa_sb = const.tile([128, L * 2], F16)
            for l in range(L):
                nc.sync.dma_start(w_sb[:, l * 128:(l + 1) * 128], w_in[l, :, :])
                nc.sync.dma_start(wa_sb[:, l * 2:(l + 1) * 2], wa_in[l, :, :])

            znT = npool.tile([128, NLOC], F16, tag="znT")
            nc.sync.dma_start(znT[:], x_in[:, :])

            for l in range(L):
                # ---------------- node phase: h, asrc/adst, table build ----
                hT = hpool.tile([128, NPAD], F16, tag="hT")
                if NPAD > NLOC:
                    nc.vector.memset(hT[:, NLOC:NPAD], 0.0)
                avb = dpool.tile([2, NPAD], F32, tag="avb")
                nc.sync.dma_start(avb[:2, NLOC:NPAD], negbig[:2, :NPAD - NLOC])
                for j in range(NCHUNK):
                    a, bnd = j * 512, min((j + 1) * 512, NLOC)
                    w_ = bnd - a
                    ph = pp.tile([128, 512], F32, tag="ph")
                    nc.tensor.matmul(ph[:, :w_], w_sb[:, l * 128:(l + 1) * 128],
                                     znT[:, a:bnd], start=True, stop=True)
                    nc.vector.tensor_copy(hT[:, a:bnd], ph[:, :w_])
                    pa = pp.tile([2, 512], F32, tag="pa")
                    nc.tensor.matmul(pa[:2, :w_], wa_sb[:, l * 2:(l + 1) * 2],
                                     znT[:, a:bnd], start=True, stop=True)
                    avc = apool.tile([2, 512], F32, tag="avc")
                    nc.vector.tensor_copy(avc[:2, :w_], pa[:2, :w_])
                    nc.sync.dma_start(avb[:2, a:bnd], avc[:2, :w_])
                asrc_g = npool.tile([128, G], F32, tag="asrc_g")
                adst_g = npool.tile([128, G], F32, tag="adst_g")
                nc.sync.dma_start(
                    asrc_g[:], avb[0, :].rearrange("(g p) -> p g", p=128))
                nc.sync.dma_start(
                    adst_g[:], avb[1, :].rearrange("(g p) -> p g", p=128))

                # table rows: transpose h per group, cast fp16, add asrc col
                stag = dpool.tile([NPAD, ROWE], F16, tag="stag")
                for g in range(G):
                    pt = ppt.tile([128, 128], F16, tag="pt")
                    nc.tensor.matmul(pt[:], hT[:, g * 128:(g + 1) * 128],
                                     ident[:], is_transpose=True,
                                     start=True, stop=True)
                    rb = apool.tile([128, 132], F16, tag="rb")
                    nc.vector.tensor_copy(rb[:, 0:128], pt[:])
                    rb32 = rb[:].bitcast(F32)  # [128, 66]
                    nc.vector.tensor_copy(rb32[:, 64:65], asrc_g[:, g:g + 1])
                    nc.vector.memset(rb32[:, 65:66], 0.0)
                    nc.sync.dma_start(stag[g * 128:(g + 1) * 128, 0:132],
                                      rb[:])
                table = dtab.tile([TROWS, ROWE], F16, tag="table")
                nc.gpsimd.collective_compute(
                    "AllGather", OP.bypass, replica_groups=rg,
                    ins=[stag[:, :]], outs=[table[:, :]])
                if debug and l == 0:
                    nc.sync.dma_start(dbg["hT"][:, :], hT[:])
                    tbs = npool.tile([128, G * NCORES, 132], F16, tag="tbs")
                    nc.sync.dma_start(
                        tbs[:], table[:, 0:132].rearrange("(g p) e -> p g e",
                                                          p=128))
                    nc.sync.dma_start(
                        dbg["table"][:, 0:132].rearrange("(g p) e -> p g e",
                                                         p=128), tbs[:])

                # ---------------- edge phase ------------------------------
                zaggT = zgpool.tile([128, NPAD], F32, tag="zaggT")
                qctr = 0
                for g in range(G):
                    kl, kh = Klo[g], Khi[g]
                    K = kl + kh
                    o = offs[g]
                    idxt = ipool.tile([128, 8 * K], I16, tag="idxt")
                    nc.scalar.dma_start(idxt[:],
                                        idx_in[:, 8 * o:8 * (o + K)])
                    gt = gpool.tile([128, K, ROWE], F16, tag="gt")
                    if _os2.environ.get("KBISECT") == "2":
                        nc.vector.memset(gt[:], 0.0)
                    # firmware ring limit: keep gathers at <=1024 indices
                    SMAX = 8
                    for (base, cnt, toff) in ([] if _os2.environ.get("KBISECT") == "2" else [(0, kl, 0), (kl, kh, 0)]):
                        tb = table[0:HALF, :] if base == 0 else \
                            table[HALF:TROWS, :]
                        for s0 in range(0, cnt, SMAX):
                            s1 = min(s0 + SMAX, cnt)
                            nc.gpsimd.dma_gather(
                                gt[:, base + s0:base + s1, :], tb,
                                idxt[:, 8 * (base + s0):8 * (base + s1)],
                                128 * (s1 - s0), 128 * (s1 - s0), ROWE,
                                queue_num=qctr % 4)
                            qctr += 1

                    import os
                    if os.environ.get("KBISECT") == "1":
                        zt = zpool.tile([128, 128], F16, tag="zt")
                        nc.vector.tensor_copy(zt[:], gt[:, 0, 0:128])
                        pz = ppt.tile([128, 128], F16, tag="pt")
                        nc.tensor.matmul(pz[:], zt[:], ident[:],
                                         is_transpose=True, start=True,
                                         stop=True)
                        nc.vector.tensor_copy(
                            zaggT[:, g * 128:(g + 1) * 128], pz[:])
                        continue
                    gt32 = gt[:].bitcast(F32)  # [128, K, 128]
                    u = spool.tile([128, K], F32, tag="u")
                    nc.vector.tensor_scalar(
                        u[:], gt32[:, :, ASRC_F32_COL:ASRC_F32_COL + 1].squeeze(-1),
                        adst_g[:, g:g + 1], None, op0=OP.add)
                    u2 = spool.tile([128, K], F32, tag="u2")
                    nc.vector.tensor_scalar_mul(u2[:], u[:], SLOPE)
                    e = spool.tile([128, K], F32, tag="e")
                    nc.vector.tensor_tensor(e[:], u[:], u2[:], OP.max)
                    mneg = spool.tile([128, 1], F32, tag="mneg")
                    nc.vector.tensor_reduce(mneg[:], e[:], axis=AX.X, op=OP.max,
                                            negate=True)
                    p16 = spool.tile([128, K], F16, tag="p16")
                    s = spool.tile([128, 1], F32, tag="s")
                    nc.scalar.activation(p16[:], e[:], AF.Exp,
                                         bias=mneg[:, 0:1], scale=1.0,
                                         accum_out=s[:, 0:1])
                    rs = spool.tile([128, 1], F32, tag="rs")
                    nc.vector.reciprocal(rs[:], s[:])
                    pn = spool.tile([128, K], F16, tag="pn")
                    nc.vector.tensor_scalar(pn[:], p16[:], rs[:, 0:1], None,
                                            op0=OP.mult)

                    nc.vector.tensor_tensor(
                        gt[:, :, 0:128], gt[:, :, 0:128],
                        pn[:].unsqueeze(-1).broadcast_to((128, K, 128)), OP.mult)
                    # pairwise-tree sum over slots (contiguous adds beat a
                    # strided tensor_reduce ~5x here)
                    zt = zpool.tile([128, 128], F16, tag="zt")
                    cur = K
                    while cur > 2:
                        hv = cur // 2
                        nc.vector.tensor_tensor(
                            gt[:, 0:hv, 0:128], gt[:, 0:hv, 0:128],
                            gt[:, cur - hv:cur, 0:128], OP.add)
                        cur = cur - hv
                    nc.vector.tensor_tensor(zt[:], gt[:, 0, 0:128],
                                            gt[:, 1, 0:128], OP.add)
                    pz = ppt.tile([128, 128], F16, tag="pt")
                    nc.tensor.matmul(pz[:], zt[:], ident[:],
                                     is_transpose=True, start=True, stop=True)
                    nc.vector.tensor_copy(zaggT[:, g * 128:(g + 1) * 128], pz[:])
                    if debug and l == 0 and g == 0:
                        nc.sync.dma_start(dbg["gt0"][:, :, :], gt[:])
                        nc.sync.dma_start(dbg["u0"][:, :], u[:])
                        nc.sync.dma_start(dbg["s0"][:, :], s[:])
                        nc.sync.dma_start(dbg["zt0"][:, :], zt[:])

                # ---------------- BN + ReLU -------------------------------
                if debug and l == 0:
                    nc.sync.dma_start(dbg["zagg"][:, :], zaggT[:])
                stats = npool.tile([128, 2], F32, tag="stats")
                nc.vector.tensor_reduce(stats[:, 0:1], zaggT[:, :NLOC],
                                        axis=AX.X, op=OP.add)
                sqp = npool.tile([128, NCHUNK], F32, tag="sqp")
                for j in range(NCHUNK):
                    a, bnd = j * 512, min((j + 1) * 512, NLOC)
                    w_ = bnd - a
                    scr = pp.tile([128, 512], F32, tag="ph")
                    nc.vector.scalar_tensor_tensor(
                        scr[:, :w_], zaggT[:, a:bnd], 0.0, zaggT[:, a:bnd],
                        op0=OP.add, op1=OP.mult,
                        accum_out=sqp[:, j:j + 1])
                nc.vector.tensor_reduce(stats[:, 1:2], sqp[:], axis=AX.X,
                                        op=OP.add)

                stb = dpool.tile([128, 2], F32, tag="stb")
                nc.sync.dma_start(stb[:, :], stats[:])
                nc.gpsimd.collective_compute(
                    "AllReduce", OP.add, replica_groups=rg,
                    ins=[stb[:, :]], outs=[stb[:, :]])
                gstats = npool.tile([128, 2], F32, tag="gstats")
                nc.sync.dma_start(gstats[:], stb[:, :])

                mu = npool.tile([128, 1], F32, tag="mu")
                nc.vector.tensor_scalar_mul(mu[:], gstats[:, 0:1],
                                            1.0 / (NLOC * NCORES))
                msq = npool.tile([128, 1], F32, tag="msq")
                nc.vector.tensor_scalar_mul(msq[:], gstats[:, 1:2],
                                            1.0 / (NLOC * NCORES))
                mu2 = npool.tile([128, 1], F32, tag="mu2")
                nc.vector.tensor_tensor(mu2[:], mu[:], mu[:], OP.mult)
                var = npool.tile([128, 1], F32, tag="var")
                nc.vector.scalar_tensor_tensor(var[:], msq[:], EPS, mu2[:],
                                               op0=OP.add, op1=OP.subtract)
                sd = npool.tile([128, 1], F32, tag="sd")
                nc.scalar.activation(sd[:], var[:], AF.Sqrt,
                                     bias=zeros1[:, 0:1], scale=1.0)
                rstd = npool.tile([128, 1], F32, tag="rstd")
                nc.vector.reciprocal(rstd[:], sd[:])
                nmr = npool.tile([128, 1], F32, tag="nmr")
                nc.vector.scalar_tensor_tensor(nmr[:], mu[:], -1.0, rstd[:],
                                               op0=OP.mult, op1=OP.mult)
                zdt = F16 if l < L - 1 else F32
                zn2 = npool.tile([128, NLOC], zdt,
                                 tag="znT" if l < L - 1 else "znTf")
                nc.scalar.activation(zn2[:], zaggT[:, :NLOC], AF.Relu,
                                     bias=nmr[:, 0:1], scale=rstd[:, 0:1])
                znT = zn2

            nc.sync.dma_start(out_t[:, :], znT[:])

    nc.compile()
    return nc


_CACHE = {}


def _get_program(key, *args, **kw):
    if key not in _CACHE:
        _CACHE[key] = _build_program(*args, **kw)
    return _CACHE[key]


def kernel(x, edge_index, W, a_src, a_dst, b):
    x = np.asarray(x, np.float32)
    edge_index = np.asarray(edge_index)
    W = np.asarray(W, np.float32)
    a_src = np.asarray(a_src, np.float32)
    a_dst = np.asarray(a_dst, np.float32)

    hp = _build_host(x, edge_index, W, a_src, a_dst)
    NLOC, G, NPAD, TOTK = hp["NLOC"], hp["G"], hp["NPAD"], hp["TOTK"]
    key = (NLOC, G, tuple(hp["Klo"]), tuple(hp["Khi"]))
    nc = _get_program(key, NLOC, G, NPAD, hp["HALF"], hp["Klo"], hp["Khi"],
                      hp["offs"], TOTK)

    ident = np.eye(128, dtype=np.float16)
    W16 = W.astype(np.float16)
    in_maps = []
    for c in range(NCORES):
        in_maps.append({
            "xt": hp["xt_maps"][c],
            "w": W16,
            "wa": hp["Wa"],
            "idx": hp["idx_maps"][c],
            "ident": ident,
        })

    res = run_bass_kernel_spmd(nc, in_maps, core_ids=list(range(NCORES)))

    N = x.shape[0]
    out = np.empty((N, 128), np.float32)
    for c in range(NCORES):
        zc = res.results[c]["zout"]  # [128, NLOC]
        out[c * NLOC + hp["perms"][c]] = zc.T
    return out


def _install_ntff_hook():
    """Make trace=True work when antenv.axon_hooks is absent (agent image)."""
    import sys as _sys
    import types as _types
    try:
        from antenv.axon_hooks import get_axon_ntff_profile_hook  # noqa: F401
        return
    except ImportError:
        pass
    try:
        import trn_agent_boot.trn_boot as _tb
        hook = _tb._ntff_profile_via_ctypes("/opt/axon/libaxon_pjrt.so")
        mod = _types.ModuleType("antenv.axon_hooks")
        mod.get_axon_ntff_profile_hook = lambda: hook
        mod.set_axon_ntff_profile_hook = lambda h: None
        _sys.modules["antenv.axon_hooks"] = mod
        import concourse.bass_utils as _bu
        _bu.upload_artifacts = lambda tmpdir: tmpdir
    except Exception:
        pass


def profile_exec_ns(inputs):
    """Run once with tracing and return HW exec time in ns (or None)."""
    _install_ntff_hook()
    x = np.asarray(inputs["x"], np.float32)
    hp = _build_host(x, np.asarray(inputs["edge_index"]),
                     np.asarray(inputs["W"], np.float32),
                     np.asarray(inputs["a_src"], np.float32),
                     np.asarray(inputs["a_dst"], np.float32))
    key = (hp["NLOC"], hp["G"], tuple(hp["Klo"]), tuple(hp["Khi"]))
    nc = _get_program(key, hp["NLOC"], hp["G"], hp["NPAD"], hp["HALF"],
                      hp["Klo"], hp["Khi"], hp["offs"], hp["TOTK"])
    ident = np.eye(128, dtype=np.float16)
    in_maps = [{"xt": hp["xt_maps"][c],
                "w": np.asarray(inputs["W"], np.float16),
                "wa": hp["Wa"], "idx": hp["idx_maps"][c], "ident": ident}
               for c in range(NCORES)]
    try:
        res = run_bass_kernel_spmd(nc, in_maps, core_ids=list(range(NCORES)),
                                   trace=True)
        return res.exec_time_ns
    except Exception as ex:
        print("profile failed:", ex)
        return None



# revision 12
# speedup vs baseline: 551.7391x; 1.0108x over previous
"""GAT (3-layer, N=50000, E=1.6M, D=128) on 8 Trainium2 NeuronCores.

Strategy (dst-sharded ELL):
  - Nodes sharded by destination across 8 cores (6250 dst/core).
  - Per core, dsts are sorted by (in-degree from low table half, then high
    half) desc and grouped into 49 groups of 128.  Edges live in a padded
    ELL layout [128 dst, K slots] per group; the slots are split into a
    "lo" block (source rows < HALF) and a "hi" block so the int16 indices
    of dma_gather can address a 25088-row table half each.
  - Per layer each core computes h = z @ W for its shard (feature-major
    via PE), builds 512B gather rows [h fp16 x128 | asrc fp32 | junk],
    and an AllGather replicates the full table.
  - Edge phase per group: two dma_gathers fetch all slot rows; softmax
    (leaky-relu, per-dst max, exp+accum, reciprocal) is native
    per-partition work; aggregation is an in-place DVE multiply plus a
    reduce over slots; PE transposes move results to feature-major.
  - BatchNorm: free-axis reductions + a [128,2] AllReduce; normalize+ReLU
    is one ACT op.

kernel() accepts FULL inputs and returns the FULL [50000,128] output.
"""

import numpy as np

import concourse.bacc as bacc
import concourse.mybir as mybir
import concourse.tile as tile
from concourse.bass_utils import run_bass_kernel_spmd

F32 = mybir.dt.float32
F16 = mybir.dt.float16
I16 = mybir.dt.int16
AX = mybir.AxisListType
OP = mybir.AluOpType
AF = mybir.ActivationFunctionType

NCORES = 8
D = 128
L = 3
EPS = 1e-5
SLOPE = 0.2
NEG_BIG = -1e30
ROWE = 256          # fp16 elems per table row (512B): 128 h + 2 asrc + junk
ASRC_F32_COL = 64   # fp32-view column of asrc within a row


# ----------------------------------------------------------------- host prep
def _build_host(x, edge_index, W, a_src, a_dst):
    N = x.shape[0]
    NLOC = N // NCORES
    G = NLOC // 128 + 1          # always >= 1 junk row per core block
    NPAD = G * 128
    HALF = (NCORES // 2) * NPAD

    src = np.concatenate([edge_index[0], np.arange(N)]).astype(np.int64)
    dst = np.concatenate([edge_index[1], np.arange(N)]).astype(np.int64)

    # global node id -> table row needs perms first; two passes.
    # pass 1: per-core degree data and perm
    pc = []
    for c in range(NCORES):
        m = (dst >= c * NLOC) & (dst < (c + 1) * NLOC)
        s_c = src[m]
        d_c = dst[m] - c * NLOC
        deg_c = np.bincount(d_c, minlength=NLOC)
        srclo = s_c < (NCORES // 2) * NLOC   # owner core < NCORES/2
        nlo_c = np.bincount(d_c[srclo], minlength=NLOC)
        nhi_c = deg_c - nlo_c
        perm_c = np.lexsort((-nhi_c, -nlo_c))
        order = np.argsort(d_c, kind="stable")
        pc.append(dict(s=s_c[order], d=d_c[order], deg=deg_c, nlo=nlo_c,
                       nhi=nhi_c, perm=perm_c))

    tablerow = np.empty(N, np.int64)
    for c in range(NCORES):
        inv = np.empty(NLOC, np.int64)
        inv[pc[c]["perm"]] = np.arange(NLOC)
        tablerow[c * NLOC:(c + 1) * NLOC] = c * NPAD + inv

    Klo = np.zeros(G, np.int64)
    Khi = np.zeros(G, np.int64)
    for g in range(G):
        for c in range(NCORES):
            rows = pc[c]["perm"][g * 128:(g + 1) * 128]
            if len(rows):
                Klo[g] = max(Klo[g], pc[c]["nlo"][rows].max())
                Khi[g] = max(Khi[g], pc[c]["nhi"][rows].max())
    Klo = np.maximum(Klo, 1)
    Khi = np.maximum(Khi, 1)
    Kt = Klo + Khi
    offs = np.zeros(G + 1, np.int64)
    np.cumsum(Kt, out=offs[1:])
    TOTK = int(offs[-1])

    def pack16(stream):  # stream [n] int64 -> [128, n//16] int16 wrapped
        n = len(stream)
        arr = stream.reshape(n // 16, 16).T.astype(np.int16)  # [16, n/16]
        return np.tile(arr, (8, 1))

    idx_maps, mask_maps, xt_maps = [], [], []
    for c in range(NCORES):
        P = pc[c]
        starts = np.zeros(NLOC + 1, np.int64)
        np.cumsum(P["deg"], out=starts[1:])
        idx16 = np.zeros((128, 8 * TOTK), np.int16)
        JUNK = NLOC  # first junk row in each half (asrc = -1e30 on device)
        for g in range(G):
            rows = P["perm"][g * 128:(g + 1) * 128]
            kl, kh = int(Klo[g]), int(Khi[g])
            lo_st = np.full((kl, 128), JUNK, np.int64)   # slot-major [k, p]
            hi_st = np.full((kh, 128), JUNK, np.int64)
            for p, dloc in enumerate(rows):
                es = P["s"][starts[dloc]:starts[dloc] + P["deg"][dloc]]
                rs = tablerow[es]
                rlo = rs[rs < HALF]
                rhi = rs[rs >= HALF] - HALF
                lo_st[:len(rlo), p] = rlo
                hi_st[:len(rhi), p] = rhi
            o16 = 8 * offs[g]
            idx16[:, o16:o16 + 8 * kl] = pack16(lo_st.ravel())
            idx16[:, o16 + 8 * kl:o16 + 8 * (kl + kh)] = pack16(hi_st.ravel())
        idx_maps.append(idx16)
        xt_maps.append(np.ascontiguousarray(
            x[c * NLOC + P["perm"]].T.astype(np.float16)))

    Wa = np.stack(
        [np.stack([W[l] @ a_src[l], W[l] @ a_dst[l]], axis=-1) for l in range(L)]
    ).astype(np.float16)  # [L,128,2]

    return dict(N=N, NLOC=NLOC, G=G, NPAD=NPAD, HALF=HALF,
                Klo=[int(k) for k in Klo], Khi=[int(k) for k in Khi],
                offs=[int(o) for o in offs], TOTK=TOTK,
                perms=[p["perm"] for p in pc],
                idx_maps=idx_maps, xt_maps=xt_maps, Wa=Wa)


# ------------------------------------------------------------- device program
def _build_program(NLOC, G, NPAD, HALF, Klo, Khi, offs, TOTK, debug=False):
    TROWS = NCORES * NPAD
    nc = bacc.Bacc("TRN2", num_devices=NCORES, num_swdge_queues=4)
    dbg = {}
    if debug:
        K0 = Klo[0] + Khi[0]
        dbg["hT"] = nc.dram_tensor("d_hT", [128, NLOC], F32, kind="ExternalOutput")
        dbg["table"] = nc.dram_tensor("d_table", [TROWS, ROWE], F16,
                                      kind="ExternalOutput")
        dbg["gt0"] = nc.dram_tensor("d_gt0", [128, K0, ROWE], F16,
                                    kind="ExternalOutput")
        dbg["u0"] = nc.dram_tensor("d_u0", [128, K0], F32, kind="ExternalOutput")
        dbg["s0"] = nc.dram_tensor("d_s0", [128, 1], F32, kind="ExternalOutput")
        dbg["zt0"] = nc.dram_tensor("d_zt0", [128, 128], F32, kind="ExternalOutput")
        dbg["zagg"] = nc.dram_tensor("d_zagg", [128, NPAD], F32,
                                     kind="ExternalOutput")

    x_in = nc.dram_tensor("xt", [128, NLOC], F16, kind="ExternalInput")
    w_in = nc.dram_tensor("w", [L, 128, 128], F16, kind="ExternalInput")
    wa_in = nc.dram_tensor("wa", [L, 128, 2], F16, kind="ExternalInput")
    idx_in = nc.dram_tensor("idx", [128, 8 * TOTK], I16, kind="ExternalInput")
    id_in = nc.dram_tensor("ident", [128, 128], F16, kind="ExternalInput")
    out_t = nc.dram_tensor("zout", [128, NLOC], F32, kind="ExternalOutput")

    NCHUNK = (NLOC + 511) // 512
    rg = [[i for i in range(NCORES)]]

    import os as _os2
    with tile.TileContext(nc, linearize=_os2.environ.get("KLIN") == "1") as tc:
        from contextlib import ExitStack
        with ExitStack() as ctx:
            const = ctx.enter_context(tc.tile_pool(name="const", bufs=1))
            npool = ctx.enter_context(tc.tile_pool(name="npool", bufs=2))
            hpool = ctx.enter_context(tc.tile_pool(name="hpool", bufs=1))
            apool = ctx.enter_context(tc.tile_pool(name="apool", bufs=2))
            zgpool = ctx.enter_context(tc.tile_pool(name="zgpool", bufs=1))
            spool = ctx.enter_context(tc.tile_pool(name="spool", bufs=4))
            gpool = ctx.enter_context(tc.tile_pool(name="gpool", bufs=2))
            ipool = ctx.enter_context(tc.tile_pool(name="ipool", bufs=6))
            zpool = ctx.enter_context(tc.tile_pool(name="zpool", bufs=2))
            pp = ctx.enter_context(tc.tile_pool(name="pp", bufs=2, space="PSUM"))
            ppt = ctx.enter_context(tc.tile_pool(name="ppt", bufs=2, space="PSUM"))
            dpool = ctx.enter_context(tc.tile_pool(name="dpool", bufs=2, space="DRAM"))
            dtab = ctx.enter_context(tc.tile_pool(name="dtab", bufs=2, space="DRAM"))

            ident = const.tile([128, 128], F16)
            nc.sync.dma_start(ident[:], id_in[:, :])
            zeros1 = const.tile([128, 1], F32)
            nc.vector.memset(zeros1[:], 0.0)
            negbig = const.tile([2, 128], F32)
            nc.vector.memset(negbig[:], NEG_BIG)
            w_sb = const.tile([128, L * 128], F16)
            wa_sb = const.tile([128, L * 2], F16)
            for l in range(L):
                nc.sync.dma_start(w_sb[:, l * 128:(l + 1) * 128], w_in[l, :, :])
                nc.sync.dma_start(wa_sb[:, l * 2:(l + 1) * 2], wa_in[l, :, :])

            znT = npool.tile([128, NLOC], F16, tag="znT")
            nc.sync.dma_start(znT[:], x_in[:, :])

            for l in range(L):
                # ---------------- node phase: h, asrc/adst, table build ----
                hT = hpool.tile([128, NPAD], F16, tag="hT")
                if NPAD > NLOC:
                    nc.vector.memset(hT[:, NLOC:NPAD], 0.0)
                avb = dpool.tile([2, NPAD], F32, tag="avb")
                nc.sync.dma_start(avb[:2, NLOC:NPAD], negbig[:2, :NPAD - NLOC])
                for j in range(NCHUNK):
                    a, bnd = j * 512, min((j + 1) * 512, NLOC)
                    w_ = bnd - a
                    ph = pp.tile([128, 512], F32, tag="ph")
                    nc.tensor.matmul(ph[:, :w_], w_sb[:, l * 128:(l + 1) * 128],
                                     znT[:, a:bnd], start=True, stop=True)
                    nc.vector.tensor_copy(hT[:, a:bnd], ph[:, :w_])
                    pa = pp.tile([2, 512], F32, tag="pa")
                    nc.tensor.matmul(pa[:2, :w_], wa_sb[:, l * 2:(l + 1) * 2],
                                     znT[:, a:bnd], start=True, stop=True)
                    avc = apool.tile([2, 512], F32, tag="avc")
                    nc.vector.tensor_copy(avc[:2, :w_], pa[:2, :w_])
                    nc.sync.dma_start(avb[:2, a:bnd], avc[:2, :w_])
                asrc_g = npool.tile([128, G], F32, tag="asrc_g")
                adst_g = npool.tile([128, G], F32, tag="adst_g")
                nc.sync.dma_start(
                    asrc_g[:], avb[0, :].rearrange("(g p) -> p g", p=128))
                nc.sync.dma_start(
                    adst_g[:], avb[1, :].rearrange("(g p) -> p g", p=128))

                # table rows: transpose h per group, cast fp16, add asrc col
                stag = dpool.tile([NPAD, ROWE], F16, tag="stag")
                for g in range(G):
                    pt = ppt.tile([128, 128], F16, tag="pt")
                    nc.tensor.matmul(pt[:], hT[:, g * 128:(g + 1) * 128],
                                     ident[:], is_transpose=True,
                                     start=True, stop=True)
                    rb = apool.tile([128, 132], F16, tag="rb")
                    nc.vector.tensor_copy(rb[:, 0:128], pt[:])
                    rb32 = rb[:].bitcast(F32)  # [128, 66]
                    nc.vector.tensor_copy(rb32[:, 64:65], asrc_g[:, g:g + 1])
                    nc.vector.memset(rb32[:, 65:66], 0.0)
                    nc.sync.dma_start(stag[g * 128:(g + 1) * 128, 0:132],
                                      rb[:])
                table = dtab.tile([TROWS, ROWE], F16, tag="table")
                nc.gpsimd.collective_compute(
                    "AllGather", OP.bypass, replica_groups=rg,
                    ins=[stag[:, :]], outs=[table[:, :]])
                if debug and l == 0:
                    nc.sync.dma_start(dbg["hT"][:, :], hT[:])
                    tbs = npool.tile([128, G * NCORES, 132], F16, tag="tbs")
                    nc.sync.dma_start(
                        tbs[:], table[:, 0:132].rearrange("(g p) e -> p g e",
                                                          p=128))
                    nc.sync.dma_start(
                        dbg["table"][:, 0:132].rearrange("(g p) e -> p g e",
                                                         p=128), tbs[:])

                # ---------------- edge phase ------------------------------
                zaggT = zgpool.tile([128, NPAD], F32, tag="zaggT")
                qctr = 0
                for g in range(G):
                    kl, kh = Klo[g], Khi[g]
                    K = kl + kh
                    o = offs[g]
                    idxt = ipool.tile([128, 8 * K], I16, tag="idxt")
                    nc.scalar.dma_start(idxt[:],
                                        idx_in[:, 8 * o:8 * (o + K)])
                    gt = gpool.tile([128, K, ROWE], F16, tag="gt")
                    if _os2.environ.get("KBISECT") == "2":
                        nc.vector.memset(gt[:], 0.0)
                    # firmware ring limit: keep gathers at <=1024 indices
                    SMAX = 8
                    for (base, cnt, toff) in ([] if _os2.environ.get("KBISECT") == "2" else [(0, kl, 0), (kl, kh, 0)]):
                        tb = table[0:HALF, :] if base == 0 else \
                            table[HALF:TROWS, :]
                        for s0 in range(0, cnt, SMAX):
                            s1 = min(s0 + SMAX, cnt)
                            nc.gpsimd.dma_gather(
                                gt[:, base + s0:base + s1, :], tb,
                                idxt[:, 8 * (base + s0):8 * (base + s1)],
                                128 * (s1 - s0), 128 * (s1 - s0), ROWE,
                                queue_num=qctr % 4)
                            qctr += 1

                    import os
                    if os.environ.get("KBISECT") == "1":
                        zt = zpool.tile([128, 128], F16, tag="zt")
                        nc.vector.tensor_copy(zt[:], gt[:, 0, 0:128])
                        pz = ppt.tile([128, 128], F16, tag="pt")
                        nc.tensor.matmul(pz[:], zt[:], ident[:],
                                         is_transpose=True, start=True,
                                         stop=True)
                        nc.vector.tensor_copy(
                            zaggT[:, g * 128:(g + 1) * 128], pz[:])
                        continue
                    gt32 = gt[:].bitcast(F32)  # [128, K, 128]
                    u = spool.tile([128, K], F32, tag="u")
                    nc.vector.tensor_scalar(
                        u[:], gt32[:, :, ASRC_F32_COL:ASRC_F32_COL + 1].squeeze(-1),
                        adst_g[:, g:g + 1], None, op0=OP.add)
                    e = spool.tile([128, K], F32, tag="e")
                    nc.vector.scalar_tensor_tensor(e[:], u[:], SLOPE, u[:],
                                                   op0=OP.mult, op1=OP.max)
                    mneg = spool.tile([128, 1], F32, tag="mneg")
                    nc.vector.tensor_reduce(mneg[:], e[:], axis=AX.X, op=OP.max,
                                            negate=True)
                    p16 = spool.tile([128, K], F16, tag="p16")
                    s = spool.tile([128, 1], F32, tag="s")
                    nc.scalar.activation(p16[:], e[:], AF.Exp,
                                         bias=mneg[:, 0:1], scale=1.0,
                                         accum_out=s[:, 0:1])
                    rs = spool.tile([128, 1], F32, tag="rs")
                    nc.vector.reciprocal(rs[:], s[:])
                    pn = spool.tile([128, K], F16, tag="pn")
                    nc.vector.tensor_scalar(pn[:], p16[:], rs[:, 0:1], None,
                                            op0=OP.mult)

                    nc.vector.tensor_tensor(
                        gt[:, :, 0:128], gt[:, :, 0:128],
                        pn[:].unsqueeze(-1).broadcast_to((128, K, 128)), OP.mult)
                    # pairwise-tree sum over slots (contiguous adds beat a
                    # strided tensor_reduce ~5x here)
                    zt = zpool.tile([128, 128], F16, tag="zt")
                    cur = K
                    while cur > 2:
                        hv = cur // 2
                        nc.vector.tensor_tensor(
                            gt[:, 0:hv, 0:128], gt[:, 0:hv, 0:128],
                            gt[:, cur - hv:cur, 0:128], OP.add)
                        cur = cur - hv
                    nc.vector.tensor_tensor(zt[:], gt[:, 0, 0:128],
                                            gt[:, 1, 0:128], OP.add)
                    pz = ppt.tile([128, 128], F16, tag="pt")
                    nc.tensor.matmul(pz[:], zt[:], ident[:],
                                     is_transpose=True, start=True, stop=True)
                    nc.vector.tensor_copy(zaggT[:, g * 128:(g + 1) * 128], pz[:])
                    if debug and l == 0 and g == 0:
                        nc.sync.dma_start(dbg["gt0"][:, :, :], gt[:])
                        nc.sync.dma_start(dbg["u0"][:, :], u[:])
                        nc.sync.dma_start(dbg["s0"][:, :], s[:])
                        nc.sync.dma_start(dbg["zt0"][:, :], zt[:])

                # ---------------- BN + ReLU -------------------------------
                if debug and l == 0:
                    nc.sync.dma_start(dbg["zagg"][:, :], zaggT[:])
                stats = npool.tile([128, 2], F32, tag="stats")
                nc.vector.tensor_reduce(stats[:, 0:1], zaggT[:, :NLOC],
                                        axis=AX.X, op=OP.add)
                sqp = npool.tile([128, NCHUNK], F32, tag="sqp")
                for j in range(NCHUNK):
                    a, bnd = j * 512, min((j + 1) * 512, NLOC)
                    w_ = bnd - a
                    scr = pp.tile([128, 512], F32, tag="ph")
                    nc.vector.scalar_tensor_tensor(
                        scr[:, :w_], zaggT[:, a:bnd], 0.0, zaggT[:, a:bnd],
                        op0=OP.add, op1=OP.mult,
                        accum_out=sqp[:, j:j + 1])
                nc.vector.tensor_reduce(stats[:, 1:2], sqp[:], axis=AX.X,
                                        op=OP.add)

                stb = dpool.tile([128, 2], F32, tag="stb")
                nc.sync.dma_start(stb[:, :], stats[:])
                nc.gpsimd.collective_compute(
                    "AllReduce", OP.add, replica_groups=rg,
                    ins=[stb[:, :]], outs=[stb[:, :]])
                gstats = npool.tile([128, 2], F32, tag="gstats")
                nc.sync.dma_start(gstats[:], stb[:, :])

                mu = npool.tile([128, 1], F32, tag="mu")
                nc.vector.tensor_scalar_mul(mu[:], gstats[:, 0:1],
                                            1.0 / (NLOC * NCORES))
                msq = npool.tile([128, 1], F32, tag="msq")
                nc.vector.tensor_scalar_mul(msq[:], gstats[:, 1:2],
                                            1.0 / (NLOC * NCORES))
                mu2 = npool.tile([128, 1], F32, tag="mu2")
                nc.vector.tensor_tensor(mu2[:], mu[:], mu[:], OP.mult)
                var = npool.tile([128, 1], F32, tag="var")
                nc.vector.scalar_tensor_tensor(var[:], msq[:], EPS, mu2[:],
                                               op0=OP.add, op1=OP.subtract)
                sd = npool.tile([128, 1], F32, tag="sd")
                nc.scalar.activation(sd[:], var[:], AF.Sqrt,
                                     bias=zeros1[:, 0:1], scale=1.0)
                rstd = npool.tile([128, 1], F32, tag="rstd")
                nc.vector.reciprocal(rstd[:], sd[:])
                nmr = npool.tile([128, 1], F32, tag="nmr")
                nc.vector.scalar_tensor_tensor(nmr[:], mu[:], -1.0, rstd[:],
                                               op0=OP.mult, op1=OP.mult)
                zdt = F16 if l < L - 1 else F32
                zn2 = npool.tile([128, NLOC], zdt,
                                 tag="znT" if l < L - 1 else "znTf")
                nc.scalar.activation(zn2[:], zaggT[:, :NLOC], AF.Relu,
                                     bias=nmr[:, 0:1], scale=rstd[:, 0:1])
                znT = zn2

            nc.sync.dma_start(out_t[:, :], znT[:])

    nc.compile()
    return nc


_CACHE = {}


def _get_program(key, *args, **kw):
    if key not in _CACHE:
        _CACHE[key] = _build_program(*args, **kw)
    return _CACHE[key]


def kernel(x, edge_index, W, a_src, a_dst, b):
    x = np.asarray(x, np.float32)
    edge_index = np.asarray(edge_index)
    W = np.asarray(W, np.float32)
    a_src = np.asarray(a_src, np.float32)
    a_dst = np.asarray(a_dst, np.float32)

    hp = _build_host(x, edge_index, W, a_src, a_dst)
    NLOC, G, NPAD, TOTK = hp["NLOC"], hp["G"], hp["NPAD"], hp["TOTK"]
    key = (NLOC, G, tuple(hp["Klo"]), tuple(hp["Khi"]))
    nc = _get_program(key, NLOC, G, NPAD, hp["HALF"], hp["Klo"], hp["Khi"],
                      hp["offs"], TOTK)

    ident = np.eye(128, dtype=np.float16)
    W16 = W.astype(np.float16)
    in_maps = []
    for c in range(NCORES):
        in_maps.append({
            "xt": hp["xt_maps"][c],
            "w": W16,
            "wa": hp["Wa"],
            "idx": hp["idx_maps"][c],
            "ident": ident,
        })

    res = run_bass_kernel_spmd(nc, in_maps, core_ids=list(range(NCORES)))

    N = x.shape[0]
    out = np.empty((N, 128), np.float32)
    for c in range(NCORES):
        zc = res.results[c]["zout"]  # [128, NLOC]
        out[c * NLOC + hp["perms"][c]] = zc.T
    return out


def _install_ntff_hook():
    """Make trace=True work when antenv.axon_hooks is absent (agent image)."""
    import sys as _sys
    import types as _types
    try:
        from antenv.axon_hooks import get_axon_ntff_profile_hook  # noqa: F401
        return
    except ImportError:
        pass
    try:
        import trn_agent_boot.trn_boot as _tb
        hook = _tb._ntff_profile_via_ctypes("/opt/axon/libaxon_pjrt.so")
        mod = _types.ModuleType("antenv.axon_hooks")
        mod.get_axon_ntff_profile_hook = lambda: hook
        mod.set_axon_ntff_profile_hook = lambda h: None
        _sys.modules["antenv.axon_hooks"] = mod
        import concourse.bass_utils as _bu
        _bu.upload_artifacts = lambda tmpdir: tmpdir
    except Exception:
        pass


def profile_exec_ns(inputs):
    """Run once with tracing and return HW exec time in ns (or None)."""
    _install_ntff_hook()
    x = np.asarray(inputs["x"], np.float32)
    hp = _build_host(x, np.asarray(inputs["edge_index"]),
                     np.asarray(inputs["W"], np.float32),
                     np.asarray(inputs["a_src"], np.float32),
                     np.asarray(inputs["a_dst"], np.float32))
    key = (hp["NLOC"], hp["G"], tuple(hp["Klo"]), tuple(hp["Khi"]))
    nc = _get_program(key, hp["NLOC"], hp["G"], hp["NPAD"], hp["HALF"],
                      hp["Klo"], hp["Khi"], hp["offs"], hp["TOTK"])
    ident = np.eye(128, dtype=np.float16)
    in_maps = [{"xt": hp["xt_maps"][c],
                "w": np.asarray(inputs["W"], np.float16),
                "wa": hp["Wa"], "idx": hp["idx_maps"][c], "ident": ident}
               for c in range(NCORES)]
    try:
        res = run_bass_kernel_spmd(nc, in_maps, core_ids=list(range(NCORES)),
                                   trace=True)
        return res.exec_time_ns
    except Exception as ex:
        print("profile failed:", ex)
        return None



# revision 13
# speedup vs baseline: 598.7171x; 1.0851x over previous
"""GAT (3-layer, N=50000, E=1.6M, D=128) on 8 Trainium2 NeuronCores.

Strategy (dst-sharded ELL):
  - Nodes sharded by destination across 8 cores (6250 dst/core).
  - Per core, dsts are sorted by (in-degree from low table half, then high
    half) desc and grouped into 49 groups of 128.  Edges live in a padded
    ELL layout [128 dst, K slots] per group; the slots are split into a
    "lo" block (source rows < HALF) and a "hi" block so the int16 indices
    of dma_gather can address a 25088-row table half each.
  - Per layer each core computes h = z @ W for its shard (feature-major
    via PE), builds 512B gather rows [h fp16 x128 | asrc fp32 | junk],
    and an AllGather replicates the full table.
  - Edge phase per group: two dma_gathers fetch all slot rows; softmax
    (leaky-relu, per-dst max, exp+accum, reciprocal) is native
    per-partition work; aggregation is an in-place DVE multiply plus a
    reduce over slots; PE transposes move results to feature-major.
  - BatchNorm: free-axis reductions + a [128,2] AllReduce; normalize+ReLU
    is one ACT op.

kernel() accepts FULL inputs and returns the FULL [50000,128] output.
"""

import numpy as np

import concourse.bacc as bacc
import concourse.mybir as mybir
import concourse.tile as tile
from concourse.bass_utils import run_bass_kernel_spmd

F32 = mybir.dt.float32
F16 = mybir.dt.float16
I16 = mybir.dt.int16
AX = mybir.AxisListType
OP = mybir.AluOpType
AF = mybir.ActivationFunctionType

NCORES = 8
D = 128
L = 3
EPS = 1e-5
SLOPE = 0.2
NEG_BIG = -1e30
ROWE = 256          # fp16 elems per table row (512B): 128 h + 2 asrc + junk
ASRC_F32_COL = 64   # fp32-view column of asrc within a row


# ----------------------------------------------------------------- host prep
def _build_host(x, edge_index, W, a_src, a_dst):
    N = x.shape[0]
    NLOC = N // NCORES
    G = NLOC // 128 + 1          # always >= 1 junk row per core block
    NPAD = G * 128
    HALF = (NCORES // 2) * NPAD

    src = np.concatenate([edge_index[0], np.arange(N)]).astype(np.int64)
    dst = np.concatenate([edge_index[1], np.arange(N)]).astype(np.int64)

    # global node id -> table row needs perms first; two passes.
    # pass 1: per-core degree data and perm
    pc = []
    for c in range(NCORES):
        m = (dst >= c * NLOC) & (dst < (c + 1) * NLOC)
        s_c = src[m]
        d_c = dst[m] - c * NLOC
        deg_c = np.bincount(d_c, minlength=NLOC)
        srclo = s_c < (NCORES // 2) * NLOC   # owner core < NCORES/2
        nlo_c = np.bincount(d_c[srclo], minlength=NLOC)
        nhi_c = deg_c - nlo_c
        perm_c = np.lexsort((-nhi_c, -nlo_c))
        order = np.argsort(d_c, kind="stable")
        pc.append(dict(s=s_c[order], d=d_c[order], deg=deg_c, nlo=nlo_c,
                       nhi=nhi_c, perm=perm_c))

    tablerow = np.empty(N, np.int64)
    for c in range(NCORES):
        inv = np.empty(NLOC, np.int64)
        inv[pc[c]["perm"]] = np.arange(NLOC)
        tablerow[c * NLOC:(c + 1) * NLOC] = c * NPAD + inv

    Klo = np.zeros(G, np.int64)
    Khi = np.zeros(G, np.int64)
    for g in range(G):
        for c in range(NCORES):
            rows = pc[c]["perm"][g * 128:(g + 1) * 128]
            if len(rows):
                Klo[g] = max(Klo[g], pc[c]["nlo"][rows].max())
                Khi[g] = max(Khi[g], pc[c]["nhi"][rows].max())
    Klo = np.maximum(Klo, 1)
    Khi = np.maximum(Khi, 1)
    Kt = Klo + Khi
    offs = np.zeros(G + 1, np.int64)
    np.cumsum(Kt, out=offs[1:])
    TOTK = int(offs[-1])

    def pack16(stream):  # stream [n] int64 -> [128, n//16] int16 wrapped
        n = len(stream)
        arr = stream.reshape(n // 16, 16).T.astype(np.int16)  # [16, n/16]
        return np.tile(arr, (8, 1))

    idx_maps, mask_maps, xt_maps = [], [], []
    for c in range(NCORES):
        P = pc[c]
        starts = np.zeros(NLOC + 1, np.int64)
        np.cumsum(P["deg"], out=starts[1:])
        idx16 = np.zeros((128, 8 * TOTK), np.int16)
        JUNK = NLOC  # first junk row in each half (asrc = -1e30 on device)
        for g in range(G):
            rows = P["perm"][g * 128:(g + 1) * 128]
            kl, kh = int(Klo[g]), int(Khi[g])
            lo_st = np.full((kl, 128), JUNK, np.int64)   # slot-major [k, p]
            hi_st = np.full((kh, 128), JUNK, np.int64)
            for p, dloc in enumerate(rows):
                es = P["s"][starts[dloc]:starts[dloc] + P["deg"][dloc]]
                rs = tablerow[es]
                rlo = rs[rs < HALF]
                rhi = rs[rs >= HALF] - HALF
                lo_st[:len(rlo), p] = rlo
                hi_st[:len(rhi), p] = rhi
            o16 = 8 * offs[g]
            idx16[:, o16:o16 + 8 * kl] = pack16(lo_st.ravel())
            idx16[:, o16 + 8 * kl:o16 + 8 * (kl + kh)] = pack16(hi_st.ravel())
        idx_maps.append(idx16)
        xt_maps.append(np.ascontiguousarray(
            x[c * NLOC + P["perm"]].T.astype(np.float16)))

    Wa = np.stack(
        [np.stack([W[l] @ a_src[l], W[l] @ a_dst[l]], axis=-1) for l in range(L)]
    ).astype(np.float16)  # [L,128,2]

    return dict(N=N, NLOC=NLOC, G=G, NPAD=NPAD, HALF=HALF,
                Klo=[int(k) for k in Klo], Khi=[int(k) for k in Khi],
                offs=[int(o) for o in offs], TOTK=TOTK,
                perms=[p["perm"] for p in pc],
                idx_maps=idx_maps, xt_maps=xt_maps, Wa=Wa)


# ------------------------------------------------------------- device program
def _build_program(NLOC, G, NPAD, HALF, Klo, Khi, offs, TOTK, debug=False):
    TROWS = NCORES * NPAD
    nc = bacc.Bacc("TRN2", num_devices=NCORES, num_swdge_queues=4)
    dbg = {}
    if debug:
        K0 = Klo[0] + Khi[0]
        dbg["hT"] = nc.dram_tensor("d_hT", [128, NLOC], F32, kind="ExternalOutput")
        dbg["table"] = nc.dram_tensor("d_table", [TROWS, ROWE], F16,
                                      kind="ExternalOutput")
        dbg["gt0"] = nc.dram_tensor("d_gt0", [128, K0, ROWE], F16,
                                    kind="ExternalOutput")
        dbg["u0"] = nc.dram_tensor("d_u0", [128, K0], F32, kind="ExternalOutput")
        dbg["s0"] = nc.dram_tensor("d_s0", [128, 1], F32, kind="ExternalOutput")
        dbg["zt0"] = nc.dram_tensor("d_zt0", [128, 128], F32, kind="ExternalOutput")
        dbg["zagg"] = nc.dram_tensor("d_zagg", [128, NPAD], F32,
                                     kind="ExternalOutput")

    x_in = nc.dram_tensor("xt", [128, NLOC], F16, kind="ExternalInput")
    w_in = nc.dram_tensor("w", [L, 128, 128], F16, kind="ExternalInput")
    wa_in = nc.dram_tensor("wa", [L, 128, 2], F16, kind="ExternalInput")
    idx_in = nc.dram_tensor("idx", [128, 8 * TOTK], I16, kind="ExternalInput")
    id_in = nc.dram_tensor("ident", [128, 128], F16, kind="ExternalInput")
    out_t = nc.dram_tensor("zout", [128, NLOC], F32, kind="ExternalOutput")

    NCHUNK = (NLOC + 511) // 512
    rg = [[i for i in range(NCORES)]]

    import os as _os2
    with tile.TileContext(nc, linearize=_os2.environ.get("KLIN") == "1") as tc:
        from contextlib import ExitStack
        with ExitStack() as ctx:
            const = ctx.enter_context(tc.tile_pool(name="const", bufs=1))
            npool = ctx.enter_context(tc.tile_pool(name="npool", bufs=2))
            hpool = ctx.enter_context(tc.tile_pool(name="hpool", bufs=1))
            apool = ctx.enter_context(tc.tile_pool(name="apool", bufs=2))
            zgpool = ctx.enter_context(tc.tile_pool(name="zgpool", bufs=1))
            spool = ctx.enter_context(tc.tile_pool(name="spool", bufs=4))
            gpool = ctx.enter_context(tc.tile_pool(name="gpool", bufs=3))
            ipool = ctx.enter_context(tc.tile_pool(name="ipool", bufs=6))
            zpool = ctx.enter_context(tc.tile_pool(name="zpool", bufs=2))
            pp = ctx.enter_context(tc.tile_pool(name="pp", bufs=2, space="PSUM"))
            ppt = ctx.enter_context(tc.tile_pool(name="ppt", bufs=2, space="PSUM"))
            dpool = ctx.enter_context(tc.tile_pool(name="dpool", bufs=2, space="DRAM"))
            dtab = ctx.enter_context(tc.tile_pool(name="dtab", bufs=2, space="DRAM"))

            ident = const.tile([128, 128], F16)
            nc.sync.dma_start(ident[:], id_in[:, :])
            zeros1 = const.tile([128, 1], F32)
            nc.vector.memset(zeros1[:], 0.0)
            negbig = const.tile([2, 128], F32)
            nc.vector.memset(negbig[:], NEG_BIG)
            w_sb = const.tile([128, L * 128], F16)
            wa_sb = const.tile([128, L * 2], F16)
            for l in range(L):
                nc.sync.dma_start(w_sb[:, l * 128:(l + 1) * 128], w_in[l, :, :])
                nc.sync.dma_start(wa_sb[:, l * 2:(l + 1) * 2], wa_in[l, :, :])

            znT = npool.tile([128, NLOC], F16, tag="znT")
            nc.sync.dma_start(znT[:], x_in[:, :])

            for l in range(L):
                # ---------------- node phase: h, asrc/adst, table build ----
                hT = hpool.tile([128, NPAD], F16, tag="hT")
                if NPAD > NLOC:
                    nc.vector.memset(hT[:, NLOC:NPAD], 0.0)
                avb = dpool.tile([2, NPAD], F32, tag="avb")
                nc.sync.dma_start(avb[:2, NLOC:NPAD], negbig[:2, :NPAD - NLOC])
                for j in range(NCHUNK):
                    a, bnd = j * 512, min((j + 1) * 512, NLOC)
                    w_ = bnd - a
                    ph = pp.tile([128, 512], F32, tag="ph")
                    nc.tensor.matmul(ph[:, :w_], w_sb[:, l * 128:(l + 1) * 128],
                                     znT[:, a:bnd], start=True, stop=True)
                    nc.vector.tensor_copy(hT[:, a:bnd], ph[:, :w_])
                    pa = pp.tile([2, 512], F32, tag="pa")
                    nc.tensor.matmul(pa[:2, :w_], wa_sb[:, l * 2:(l + 1) * 2],
                                     znT[:, a:bnd], start=True, stop=True)
                    avc = apool.tile([2, 512], F32, tag="avc")
                    nc.vector.tensor_copy(avc[:2, :w_], pa[:2, :w_])
                    nc.sync.dma_start(avb[:2, a:bnd], avc[:2, :w_])
                asrc_g = npool.tile([128, G], F32, tag="asrc_g")
                adst_g = npool.tile([128, G], F32, tag="adst_g")
                nc.sync.dma_start(
                    asrc_g[:], avb[0, :].rearrange("(g p) -> p g", p=128))
                nc.sync.dma_start(
                    adst_g[:], avb[1, :].rearrange("(g p) -> p g", p=128))

                # table rows: transpose h per group, cast fp16, add asrc col
                stag = dpool.tile([NPAD, ROWE], F16, tag="stag")
                for g in range(G):
                    pt = ppt.tile([128, 128], F16, tag="pt")
                    nc.tensor.matmul(pt[:], hT[:, g * 128:(g + 1) * 128],
                                     ident[:], is_transpose=True,
                                     start=True, stop=True)
                    rb = apool.tile([128, 132], F16, tag="rb")
                    nc.vector.tensor_copy(rb[:, 0:128], pt[:])
                    rb32 = rb[:].bitcast(F32)  # [128, 66]
                    nc.vector.tensor_copy(rb32[:, 64:65], asrc_g[:, g:g + 1])
                    nc.vector.memset(rb32[:, 65:66], 0.0)
                    nc.sync.dma_start(stag[g * 128:(g + 1) * 128, 0:132],
                                      rb[:])
                table = dtab.tile([TROWS, ROWE], F16, tag="table")
                nc.gpsimd.collective_compute(
                    "AllGather", OP.bypass, replica_groups=rg,
                    ins=[stag[:, :]], outs=[table[:, :]])
                if debug and l == 0:
                    nc.sync.dma_start(dbg["hT"][:, :], hT[:])
                    tbs = npool.tile([128, G * NCORES, 132], F16, tag="tbs")
                    nc.sync.dma_start(
                        tbs[:], table[:, 0:132].rearrange("(g p) e -> p g e",
                                                          p=128))
                    nc.sync.dma_start(
                        dbg["table"][:, 0:132].rearrange("(g p) e -> p g e",
                                                         p=128), tbs[:])

                # ---------------- edge phase ------------------------------
                zaggT = zgpool.tile([128, NPAD], F16, tag="zaggT")
                qctr = 0
                for g in range(G):
                    kl, kh = Klo[g], Khi[g]
                    K = kl + kh
                    o = offs[g]
                    idxt = ipool.tile([128, 8 * K], I16, tag="idxt")
                    nc.scalar.dma_start(idxt[:],
                                        idx_in[:, 8 * o:8 * (o + K)])
                    gt = gpool.tile([128, K, ROWE], F16, tag="gt")
                    if _os2.environ.get("KBISECT") == "2":
                        nc.vector.memset(gt[:], 0.0)
                    # firmware ring limit: keep gathers at <=1024 indices
                    SMAX = 8
                    for (base, cnt, toff) in ([] if _os2.environ.get("KBISECT") == "2" else [(0, kl, 0), (kl, kh, 0)]):
                        tb = table[0:HALF, :] if base == 0 else \
                            table[HALF:TROWS, :]
                        for s0 in range(0, cnt, SMAX):
                            s1 = min(s0 + SMAX, cnt)
                            nc.gpsimd.dma_gather(
                                gt[:, base + s0:base + s1, :], tb,
                                idxt[:, 8 * (base + s0):8 * (base + s1)],
                                128 * (s1 - s0), 128 * (s1 - s0), ROWE,
                                queue_num=qctr % 4)
                            qctr += 1

                    import os
                    if os.environ.get("KBISECT") == "1":
                        zt = zpool.tile([128, 128], F16, tag="zt")
                        nc.vector.tensor_copy(zt[:], gt[:, 0, 0:128])
                        pz = ppt.tile([128, 128], F16, tag="pt")
                        nc.tensor.matmul(pz[:], zt[:], ident[:],
                                         is_transpose=True, start=True,
                                         stop=True)
                        nc.vector.tensor_copy(
                            zaggT[:, g * 128:(g + 1) * 128], pz[:])
                        continue
                    gt32 = gt[:].bitcast(F32)  # [128, K, 128]
                    u = spool.tile([128, K], F32, tag="u")
                    nc.vector.tensor_scalar(
                        u[:], gt32[:, :, ASRC_F32_COL:ASRC_F32_COL + 1].squeeze(-1),
                        adst_g[:, g:g + 1], None, op0=OP.add)
                    e = spool.tile([128, K], F32, tag="e")
                    nc.vector.scalar_tensor_tensor(e[:], u[:], SLOPE, u[:],
                                                   op0=OP.mult, op1=OP.max)
                    mneg = spool.tile([128, 1], F32, tag="mneg")
                    nc.vector.tensor_reduce(mneg[:], e[:], axis=AX.X, op=OP.max,
                                            negate=True)
                    p16 = spool.tile([128, K], F16, tag="p16")
                    s = spool.tile([128, 1], F32, tag="s")
                    nc.scalar.activation(p16[:], e[:], AF.Exp,
                                         bias=mneg[:, 0:1], scale=1.0,
                                         accum_out=s[:, 0:1])
                    rs = spool.tile([128, 1], F32, tag="rs")
                    nc.vector.reciprocal(rs[:], s[:])
                    pn = spool.tile([128, K], F16, tag="pn")
                    nc.vector.tensor_scalar(pn[:], p16[:], rs[:, 0:1], None,
                                            op0=OP.mult)

                    nc.vector.tensor_tensor(
                        gt[:, :, 0:128], gt[:, :, 0:128],
                        pn[:].unsqueeze(-1).broadcast_to((128, K, 128)), OP.mult)
                    # pairwise-tree sum over slots (contiguous adds beat a
                    # strided tensor_reduce ~5x here)
                    zt = zpool.tile([128, 128], F16, tag="zt")
                    cur = K
                    while cur > 2:
                        hv = cur // 2
                        nc.vector.tensor_tensor(
                            gt[:, 0:hv, 0:128], gt[:, 0:hv, 0:128],
                            gt[:, cur - hv:cur, 0:128], OP.add)
                        cur = cur - hv
                    nc.vector.tensor_tensor(zt[:], gt[:, 0, 0:128],
                                            gt[:, 1, 0:128], OP.add)
                    pz = ppt.tile([128, 128], F16, tag="pt")
                    nc.tensor.matmul(pz[:], zt[:], ident[:],
                                     is_transpose=True, start=True, stop=True)
                    nc.vector.tensor_copy(zaggT[:, g * 128:(g + 1) * 128], pz[:])
                    if debug and l == 0 and g == 0:
                        nc.sync.dma_start(dbg["gt0"][:, :, :], gt[:])
                        nc.sync.dma_start(dbg["u0"][:, :], u[:])
                        nc.sync.dma_start(dbg["s0"][:, :], s[:])
                        nc.sync.dma_start(dbg["zt0"][:, :], zt[:])

                # ---------------- BN + ReLU -------------------------------
                if debug and l == 0:
                    nc.sync.dma_start(dbg["zagg"][:, :], zaggT[:])
                stats = npool.tile([128, 2], F32, tag="stats")
                nc.vector.tensor_reduce(stats[:, 0:1], zaggT[:, :NLOC],
                                        axis=AX.X, op=OP.add)
                sqp = npool.tile([128, NCHUNK], F32, tag="sqp")
                for j in range(NCHUNK):
                    a, bnd = j * 512, min((j + 1) * 512, NLOC)
                    w_ = bnd - a
                    scr = pp.tile([128, 512], F32, tag="ph")
                    nc.vector.scalar_tensor_tensor(
                        scr[:, :w_], zaggT[:, a:bnd], 0.0, zaggT[:, a:bnd],
                        op0=OP.add, op1=OP.mult,
                        accum_out=sqp[:, j:j + 1])
                nc.vector.tensor_reduce(stats[:, 1:2], sqp[:], axis=AX.X,
                                        op=OP.add)

                stb = dpool.tile([128, 2], F32, tag="stb")
                nc.sync.dma_start(stb[:, :], stats[:])
                nc.gpsimd.collective_compute(
                    "AllReduce", OP.add, replica_groups=rg,
                    ins=[stb[:, :]], outs=[stb[:, :]])
                gstats = npool.tile([128, 2], F32, tag="gstats")
                nc.sync.dma_start(gstats[:], stb[:, :])

                mu = npool.tile([128, 1], F32, tag="mu")
                nc.vector.tensor_scalar_mul(mu[:], gstats[:, 0:1],
                                            1.0 / (NLOC * NCORES))
                msq = npool.tile([128, 1], F32, tag="msq")
                nc.vector.tensor_scalar_mul(msq[:], gstats[:, 1:2],
                                            1.0 / (NLOC * NCORES))
                mu2 = npool.tile([128, 1], F32, tag="mu2")
                nc.vector.tensor_tensor(mu2[:], mu[:], mu[:], OP.mult)
                var = npool.tile([128, 1], F32, tag="var")
                nc.vector.scalar_tensor_tensor(var[:], msq[:], EPS, mu2[:],
                                               op0=OP.add, op1=OP.subtract)
                sd = npool.tile([128, 1], F32, tag="sd")
                nc.scalar.activation(sd[:], var[:], AF.Sqrt,
                                     bias=zeros1[:, 0:1], scale=1.0)
                rstd = npool.tile([128, 1], F32, tag="rstd")
                nc.vector.reciprocal(rstd[:], sd[:])
                nmr = npool.tile([128, 1], F32, tag="nmr")
                nc.vector.scalar_tensor_tensor(nmr[:], mu[:], -1.0, rstd[:],
                                               op0=OP.mult, op1=OP.mult)
                if l < L - 1:
                    zn2 = npool.tile([128, NLOC], F16, tag="znT")
                else:
                    zn2 = hpool.tile([128, NLOC], F32, tag="znTf")
                nc.scalar.activation(zn2[:], zaggT[:, :NLOC], AF.Relu,
                                     bias=nmr[:, 0:1], scale=rstd[:, 0:1])
                znT = zn2

            nc.sync.dma_start(out_t[:, :], znT[:])

    nc.compile()
    return nc


_CACHE = {}


def _get_program(key, *args, **kw):
    if key not in _CACHE:
        _CACHE[key] = _build_program(*args, **kw)
    return _CACHE[key]


def kernel(x, edge_index, W, a_src, a_dst, b):
    x = np.asarray(x, np.float32)
    edge_index = np.asarray(edge_index)
    W = np.asarray(W, np.float32)
    a_src = np.asarray(a_src, np.float32)
    a_dst = np.asarray(a_dst, np.float32)

    hp = _build_host(x, edge_index, W, a_src, a_dst)
    NLOC, G, NPAD, TOTK = hp["NLOC"], hp["G"], hp["NPAD"], hp["TOTK"]
    key = (NLOC, G, tuple(hp["Klo"]), tuple(hp["Khi"]))
    nc = _get_program(key, NLOC, G, NPAD, hp["HALF"], hp["Klo"], hp["Khi"],
                      hp["offs"], TOTK)

    ident = np.eye(128, dtype=np.float16)
    W16 = W.astype(np.float16)
    in_maps = []
    for c in range(NCORES):
        in_maps.append({
            "xt": hp["xt_maps"][c],
            "w": W16,
            "wa": hp["Wa"],
            "idx": hp["idx_maps"][c],
            "ident": ident,
        })

    res = run_bass_kernel_spmd(nc, in_maps, core_ids=list(range(NCORES)))

    N = x.shape[0]
    out = np.empty((N, 128), np.float32)
    for c in range(NCORES):
        zc = res.results[c]["zout"]  # [128, NLOC]
        out[c * NLOC + hp["perms"][c]] = zc.T
    return out


def _install_ntff_hook():
    """Make trace=True work when antenv.axon_hooks is absent (agent image)."""
    import sys as _sys
    import types as _types
    try:
        from antenv.axon_hooks import get_axon_ntff_profile_hook  # noqa: F401
        return
    except ImportError:
        pass
    try:
        import trn_agent_boot.trn_boot as _tb
        hook = _tb._ntff_profile_via_ctypes("/opt/axon/libaxon_pjrt.so")
        mod = _types.ModuleType("antenv.axon_hooks")
        mod.get_axon_ntff_profile_hook = lambda: hook
        mod.set_axon_ntff_profile_hook = lambda h: None
        _sys.modules["antenv.axon_hooks"] = mod
        import concourse.bass_utils as _bu
        _bu.upload_artifacts = lambda tmpdir: tmpdir
    except Exception:
        pass


def profile_exec_ns(inputs):
    """Run once with tracing and return HW exec time in ns (or None)."""
    _install_ntff_hook()
    x = np.asarray(inputs["x"], np.float32)
    hp = _build_host(x, np.asarray(inputs["edge_index"]),
                     np.asarray(inputs["W"], np.float32),
                     np.asarray(inputs["a_src"], np.float32),
                     np.asarray(inputs["a_dst"], np.float32))
    key = (hp["NLOC"], hp["G"], tuple(hp["Klo"]), tuple(hp["Khi"]))
    nc = _get_program(key, hp["NLOC"], hp["G"], hp["NPAD"], hp["HALF"],
                      hp["Klo"], hp["Khi"], hp["offs"], hp["TOTK"])
    ident = np.eye(128, dtype=np.float16)
    in_maps = [{"xt": hp["xt_maps"][c],
                "w": np.asarray(inputs["W"], np.float16),
                "wa": hp["Wa"], "idx": hp["idx_maps"][c], "ident": ident}
               for c in range(NCORES)]
    try:
        res = run_bass_kernel_spmd(nc, in_maps, core_ids=list(range(NCORES)),
                                   trace=True)
        return res.exec_time_ns
    except Exception as ex:
        print("profile failed:", ex)
        return None



# revision 15
# speedup vs baseline: 615.3920x; 1.0279x over previous
"""GAT (3-layer, N=50000, E=1.6M, D=128) on 8 Trainium2 NeuronCores.

Strategy (dst-sharded ELL):
  - Nodes sharded by destination across 8 cores (6250 dst/core).
  - Per core, dsts are sorted by (in-degree from low table half, then high
    half) desc and grouped into 49 groups of 128.  Edges live in a padded
    ELL layout [128 dst, K slots] per group; the slots are split into a
    "lo" block (source rows < HALF) and a "hi" block so the int16 indices
    of dma_gather can address a 25088-row table half each.
  - Per layer each core computes h = z @ W for its shard (feature-major
    via PE), builds 512B gather rows [h fp16 x128 | asrc fp32 | junk],
    and an AllGather replicates the full table.
  - Edge phase per group: dma_gathers (1024-idx calls, round-robin over
    4 SWDGE queues, triple-buffered tiles) fetch all slot rows; softmax
    (fused leaky-relu, per-dst max, exp+accum, reciprocal) is native
    per-partition work; aggregation is an in-place DVE multiply plus a
    pairwise-tree sum over slots; PE transposes move results to
    feature-major.  fp16 throughout the node path (SWDGE descriptor
    generation on the Q7 is the bottleneck at ~8ns/row).
  - BatchNorm: free-axis reductions + a [128,2] AllReduce; normalize+ReLU
    is one ACT op.

kernel() accepts FULL inputs and returns the FULL [50000,128] output.
"""

import numpy as np

import concourse.bacc as bacc
import concourse.mybir as mybir
import concourse.tile as tile
from concourse.bass_utils import run_bass_kernel_spmd

F32 = mybir.dt.float32
F16 = mybir.dt.float16
I16 = mybir.dt.int16
AX = mybir.AxisListType
OP = mybir.AluOpType
AF = mybir.ActivationFunctionType

NCORES = 8
D = 128
L = 3
EPS = 1e-5
SLOPE = 0.2
NEG_BIG = -1e30
ROWE = 256          # fp16 elems per table row (512B): 128 h + 2 asrc + junk
ASRC_F32_COL = 64   # fp32-view column of asrc within a row


# ----------------------------------------------------------------- host prep
def _build_host(x, edge_index, W, a_src, a_dst):
    N = x.shape[0]
    NLOC = N // NCORES
    G = NLOC // 128 + 1          # always >= 1 junk row per core block
    NPAD = G * 128
    HALF = (NCORES // 2) * NPAD

    src = np.concatenate([edge_index[0], np.arange(N)]).astype(np.int64)
    dst = np.concatenate([edge_index[1], np.arange(N)]).astype(np.int64)

    # global node id -> table row needs perms first; two passes.
    # pass 1: per-core degree data and perm
    pc = []
    for c in range(NCORES):
        m = (dst >= c * NLOC) & (dst < (c + 1) * NLOC)
        s_c = src[m]
        d_c = dst[m] - c * NLOC
        deg_c = np.bincount(d_c, minlength=NLOC)
        srclo = s_c < (NCORES // 2) * NLOC   # owner core < NCORES/2
        nlo_c = np.bincount(d_c[srclo], minlength=NLOC)
        nhi_c = deg_c - nlo_c
        perm_c = np.lexsort((-nhi_c, -nlo_c))
        order = np.argsort(d_c, kind="stable")
        pc.append(dict(s=s_c[order], d=d_c[order], deg=deg_c, nlo=nlo_c,
                       nhi=nhi_c, perm=perm_c))

    tablerow = np.empty(N, np.int64)
    for c in range(NCORES):
        inv = np.empty(NLOC, np.int64)
        inv[pc[c]["perm"]] = np.arange(NLOC)
        tablerow[c * NLOC:(c + 1) * NLOC] = c * NPAD + inv

    Klo = np.zeros(G, np.int64)
    Khi = np.zeros(G, np.int64)
    for g in range(G):
        for c in range(NCORES):
            rows = pc[c]["perm"][g * 128:(g + 1) * 128]
            if len(rows):
                Klo[g] = max(Klo[g], pc[c]["nlo"][rows].max())
                Khi[g] = max(Khi[g], pc[c]["nhi"][rows].max())
    Klo = np.maximum(Klo, 1)
    Khi = np.maximum(Khi, 1)
    Kt = Klo + Khi
    offs = np.zeros(G + 1, np.int64)
    np.cumsum(Kt, out=offs[1:])
    TOTK = int(offs[-1])

    def pack16(stream):  # stream [n] int64 -> [128, n//16] int16 wrapped
        n = len(stream)
        arr = stream.reshape(n // 16, 16).T.astype(np.int16)  # [16, n/16]
        return np.tile(arr, (8, 1))

    idx_maps, mask_maps, xt_maps = [], [], []
    for c in range(NCORES):
        P = pc[c]
        starts = np.zeros(NLOC + 1, np.int64)
        np.cumsum(P["deg"], out=starts[1:])
        idx16 = np.zeros((128, 8 * TOTK), np.int16)
        JUNK = NLOC  # first junk row in each half (asrc = -1e30 on device)
        for g in range(G):
            rows = P["perm"][g * 128:(g + 1) * 128]
            kl, kh = int(Klo[g]), int(Khi[g])
            lo_st = np.full((kl, 128), JUNK, np.int64)   # slot-major [k, p]
            hi_st = np.full((kh, 128), JUNK, np.int64)
            for p, dloc in enumerate(rows):
                es = P["s"][starts[dloc]:starts[dloc] + P["deg"][dloc]]
                rs = tablerow[es]
                rlo = rs[rs < HALF]
                rhi = rs[rs >= HALF] - HALF
                lo_st[:len(rlo), p] = rlo
                hi_st[:len(rhi), p] = rhi
            o16 = 8 * offs[g]
            idx16[:, o16:o16 + 8 * kl] = pack16(lo_st.ravel())
            idx16[:, o16 + 8 * kl:o16 + 8 * (kl + kh)] = pack16(hi_st.ravel())
        idx_maps.append(idx16)
        xt_maps.append(np.ascontiguousarray(
            x[c * NLOC + P["perm"]].T.astype(np.float16)))

    Wa = np.stack(
        [np.stack([W[l] @ a_src[l], W[l] @ a_dst[l]], axis=-1) for l in range(L)]
    ).astype(np.float16)  # [L,128,2]

    return dict(N=N, NLOC=NLOC, G=G, NPAD=NPAD, HALF=HALF,
                Klo=[int(k) for k in Klo], Khi=[int(k) for k in Khi],
                offs=[int(o) for o in offs], TOTK=TOTK,
                perms=[p["perm"] for p in pc],
                idx_maps=idx_maps, xt_maps=xt_maps, Wa=Wa)


# ------------------------------------------------------------- device program
def _build_program(NLOC, G, NPAD, HALF, Klo, Khi, offs, TOTK, debug=False):
    TROWS = NCORES * NPAD
    nc = bacc.Bacc("TRN2", num_devices=NCORES, num_swdge_queues=4)
    dbg = {}
    if debug:
        K0 = Klo[0] + Khi[0]
        dbg["hT"] = nc.dram_tensor("d_hT", [128, NLOC], F32, kind="ExternalOutput")
        dbg["table"] = nc.dram_tensor("d_table", [TROWS, ROWE], F16,
                                      kind="ExternalOutput")
        dbg["gt0"] = nc.dram_tensor("d_gt0", [128, K0, ROWE], F16,
                                    kind="ExternalOutput")
        dbg["u0"] = nc.dram_tensor("d_u0", [128, K0], F32, kind="ExternalOutput")
        dbg["s0"] = nc.dram_tensor("d_s0", [128, 1], F32, kind="ExternalOutput")
        dbg["zt0"] = nc.dram_tensor("d_zt0", [128, 128], F32, kind="ExternalOutput")
        dbg["zagg"] = nc.dram_tensor("d_zagg", [128, NPAD], F32,
                                     kind="ExternalOutput")

    x_in = nc.dram_tensor("xt", [128, NLOC], F16, kind="ExternalInput")
    w_in = nc.dram_tensor("w", [L, 128, 128], F16, kind="ExternalInput")
    wa_in = nc.dram_tensor("wa", [L, 128, 2], F16, kind="ExternalInput")
    idx_in = nc.dram_tensor("idx", [128, 8 * TOTK], I16, kind="ExternalInput")
    id_in = nc.dram_tensor("ident", [128, 128], F16, kind="ExternalInput")
    out_t = nc.dram_tensor("zout", [128, NLOC], F32, kind="ExternalOutput")

    NCHUNK = (NLOC + 511) // 512
    rg = [[i for i in range(NCORES)]]

    import os as _os2
    with tile.TileContext(nc, linearize=_os2.environ.get("KLIN") == "1") as tc:
        from contextlib import ExitStack
        with ExitStack() as ctx:
            const = ctx.enter_context(tc.tile_pool(name="const", bufs=1))
            npool = ctx.enter_context(tc.tile_pool(name="npool", bufs=2))
            hpool = ctx.enter_context(tc.tile_pool(name="hpool", bufs=1))
            apool = ctx.enter_context(tc.tile_pool(name="apool", bufs=2))
            zgpool = ctx.enter_context(tc.tile_pool(name="zgpool", bufs=1))
            spool = ctx.enter_context(tc.tile_pool(name="spool", bufs=4))
            gpool = ctx.enter_context(tc.tile_pool(name="gpool", bufs=3))
            ipool = ctx.enter_context(tc.tile_pool(name="ipool", bufs=6))
            zpool = ctx.enter_context(tc.tile_pool(name="zpool", bufs=2))
            pp = ctx.enter_context(tc.tile_pool(name="pp", bufs=2, space="PSUM"))
            ppt = ctx.enter_context(tc.tile_pool(name="ppt", bufs=2, space="PSUM"))
            dpool = ctx.enter_context(tc.tile_pool(name="dpool", bufs=2, space="DRAM"))
            dtab = ctx.enter_context(tc.tile_pool(name="dtab", bufs=2, space="DRAM"))

            ident = const.tile([128, 128], F16)
            nc.sync.dma_start(ident[:], id_in[:, :])
            zeros1 = const.tile([128, 1], F32)
            nc.vector.memset(zeros1[:], 0.0)
            negbig = const.tile([2, 128], F32)
            nc.vector.memset(negbig[:], NEG_BIG)
            w_sb = const.tile([128, L * 128], F16)
            wa_sb = const.tile([128, L * 2], F16)
            for l in range(L):
                nc.sync.dma_start(w_sb[:, l * 128:(l + 1) * 128], w_in[l, :, :])
                nc.sync.dma_start(wa_sb[:, l * 2:(l + 1) * 2], wa_in[l, :, :])

            znT = npool.tile([128, NLOC], F16, tag="znT")
            nc.sync.dma_start(znT[:], x_in[:, :])

            for l in range(L):
                # ---------------- node phase: h, asrc/adst, table build ----
                hT = hpool.tile([128, NPAD], F16, tag="hT")
                if NPAD > NLOC:
                    nc.vector.memset(hT[:, NLOC:NPAD], 0.0)
                avb = dpool.tile([2, NPAD], F32, tag="avb")
                nc.sync.dma_start(avb[:2, NLOC:NPAD], negbig[:2, :NPAD - NLOC])
                for j in range(NCHUNK):
                    a, bnd = j * 512, min((j + 1) * 512, NLOC)
                    w_ = bnd - a
                    ph = pp.tile([128, 512], F32, tag="ph")
                    nc.tensor.matmul(ph[:, :w_], w_sb[:, l * 128:(l + 1) * 128],
                                     znT[:, a:bnd], start=True, stop=True)
                    nc.vector.tensor_copy(hT[:, a:bnd], ph[:, :w_])
                    pa = pp.tile([2, 512], F32, tag="pa")
                    nc.tensor.matmul(pa[:2, :w_], wa_sb[:, l * 2:(l + 1) * 2],
                                     znT[:, a:bnd], start=True, stop=True)
                    avc = apool.tile([2, 512], F32, tag="avc")
                    nc.vector.tensor_copy(avc[:2, :w_], pa[:2, :w_])
                    nc.sync.dma_start(avb[:2, a:bnd], avc[:2, :w_])
                asrc_g = npool.tile([128, G], F32, tag="asrc_g")
                adst_g = npool.tile([128, G], F32, tag="adst_g")
                nc.sync.dma_start(
                    asrc_g[:], avb[0, :].rearrange("(g p) -> p g", p=128))
                nc.sync.dma_start(
                    adst_g[:], avb[1, :].rearrange("(g p) -> p g", p=128))

                # table rows: transpose h per group, cast fp16, add asrc col
                stag = dpool.tile([NPAD, ROWE], F16, tag="stag")
                for g in range(G):
                    pt = ppt.tile([128, 128], F16, tag="pt")
                    nc.tensor.matmul(pt[:], hT[:, g * 128:(g + 1) * 128],
                                     ident[:], is_transpose=True,
                                     start=True, stop=True)
                    rb = apool.tile([128, 132], F16, tag="rb")
                    nc.vector.tensor_copy(rb[:, 0:128], pt[:])
                    rb32 = rb[:].bitcast(F32)  # [128, 66]
                    nc.vector.tensor_copy(rb32[:, 64:65], asrc_g[:, g:g + 1])
                    nc.vector.memset(rb32[:, 65:66], 0.0)
                    nc.sync.dma_start(stag[g * 128:(g + 1) * 128, 0:132],
                                      rb[:])
                table = dtab.tile([TROWS, ROWE], F16, tag="table")
                nc.gpsimd.collective_compute(
                    "AllGather", OP.bypass, replica_groups=rg,
                    ins=[stag[:, :]], outs=[table[:, :]])
                if debug and l == 0:
                    nc.sync.dma_start(dbg["hT"][:, :], hT[:])
                    tbs = npool.tile([128, G * NCORES, 132], F16, tag="tbs")
                    nc.sync.dma_start(
                        tbs[:], table[:, 0:132].rearrange("(g p) e -> p g e",
                                                          p=128))
                    nc.sync.dma_start(
                        dbg["table"][:, 0:132].rearrange("(g p) e -> p g e",
                                                         p=128), tbs[:])

                # ---------------- edge phase ------------------------------
                zaggT = zgpool.tile([128, NPAD], F16, tag="zaggT")
                qctr = 0
                for g in range(G):
                    kl, kh = Klo[g], Khi[g]
                    K = kl + kh
                    o = offs[g]
                    idxt = ipool.tile([128, 8 * K], I16, tag="idxt")
                    nc.sync.dma_start(idxt[:],
                                      idx_in[:, 8 * o:8 * (o + K)])
                    gt = gpool.tile([128, K, ROWE], F16, tag="gt")
                    if _os2.environ.get("KBISECT") == "2":
                        nc.vector.memset(gt[:], 0.0)
                    # firmware ring limit: keep gathers at <=1024 indices
                    SMAX = 8
                    for (base, cnt, toff) in ([] if _os2.environ.get("KBISECT") == "2" else [(0, kl, 0), (kl, kh, 0)]):
                        tb = table[0:HALF, :] if base == 0 else \
                            table[HALF:TROWS, :]
                        for s0 in range(0, cnt, SMAX):
                            s1 = min(s0 + SMAX, cnt)
                            nc.gpsimd.dma_gather(
                                gt[:, base + s0:base + s1, :], tb,
                                idxt[:, 8 * (base + s0):8 * (base + s1)],
                                128 * (s1 - s0), 128 * (s1 - s0), ROWE,
                                queue_num=qctr % 4)
                            qctr += 1

                    import os
                    if os.environ.get("KBISECT") == "1":
                        zt = zpool.tile([128, 128], F16, tag="zt")
                        nc.vector.tensor_copy(zt[:], gt[:, 0, 0:128])
                        pz = ppt.tile([128, 128], F16, tag="pt")
                        nc.tensor.matmul(pz[:], zt[:], ident[:],
                                         is_transpose=True, start=True,
                                         stop=True)
                        nc.vector.tensor_copy(
                            zaggT[:, g * 128:(g + 1) * 128], pz[:])
                        continue
                    gt32 = gt[:].bitcast(F32)  # [128, K, 128]
                    u = spool.tile([128, K], F32, tag="u")
                    nc.vector.tensor_scalar(
                        u[:], gt32[:, :, ASRC_F32_COL:ASRC_F32_COL + 1].squeeze(-1),
                        adst_g[:, g:g + 1], None, op0=OP.add)
                    e = spool.tile([128, K], F32, tag="e")
                    nc.vector.scalar_tensor_tensor(e[:], u[:], SLOPE, u[:],
                                                   op0=OP.mult, op1=OP.max)
                    mneg = spool.tile([128, 1], F32, tag="mneg")
                    nc.vector.tensor_reduce(mneg[:], e[:], axis=AX.X, op=OP.max,
                                            negate=True)
                    p16 = spool.tile([128, K], F16, tag="p16")
                    s = spool.tile([128, 1], F32, tag="s")
                    nc.scalar.activation(p16[:], e[:], AF.Exp,
                                         bias=mneg[:, 0:1], scale=1.0,
                                         accum_out=s[:, 0:1])
                    rs = spool.tile([128, 1], F32, tag="rs")
                    nc.vector.reciprocal(rs[:], s[:])
                    pn = spool.tile([128, K], F16, tag="pn")
                    nc.vector.tensor_scalar(pn[:], p16[:], rs[:, 0:1], None,
                                            op0=OP.mult)

                    nc.vector.tensor_tensor(
                        gt[:, :, 0:128], gt[:, :, 0:128],
                        pn[:].unsqueeze(-1).broadcast_to((128, K, 128)), OP.mult)
                    # pairwise-tree sum over slots (contiguous adds beat a
                    # strided tensor_reduce ~5x here)
                    zt = zpool.tile([128, 128], F16, tag="zt")
                    cur = K
                    while cur > 2:
                        hv = cur // 2
                        nc.vector.tensor_tensor(
                            gt[:, 0:hv, 0:128], gt[:, 0:hv, 0:128],
                            gt[:, cur - hv:cur, 0:128], OP.add)
                        cur = cur - hv
                    nc.vector.tensor_tensor(zt[:], gt[:, 0, 0:128],
                                            gt[:, 1, 0:128], OP.add)
                    pz = ppt.tile([128, 128], F16, tag="pt")
                    nc.tensor.matmul(pz[:], zt[:], ident[:],
                                     is_transpose=True, start=True, stop=True)
                    nc.vector.tensor_copy(zaggT[:, g * 128:(g + 1) * 128], pz[:])
                    if debug and l == 0 and g == 0:
                        nc.sync.dma_start(dbg["gt0"][:, :, :], gt[:])
                        nc.sync.dma_start(dbg["u0"][:, :], u[:])
                        nc.sync.dma_start(dbg["s0"][:, :], s[:])
                        nc.sync.dma_start(dbg["zt0"][:, :], zt[:])

                # ---------------- BN + ReLU -------------------------------
                if debug and l == 0:
                    nc.sync.dma_start(dbg["zagg"][:, :], zaggT[:])
                stats = npool.tile([128, 2], F32, tag="stats")
                nc.vector.tensor_reduce(stats[:, 0:1], zaggT[:, :NLOC],
                                        axis=AX.X, op=OP.add)
                sqp = npool.tile([128, NCHUNK], F32, tag="sqp")
                for j in range(NCHUNK):
                    a, bnd = j * 512, min((j + 1) * 512, NLOC)
                    w_ = bnd - a
                    scr = pp.tile([128, 512], F32, tag="ph")
                    nc.vector.scalar_tensor_tensor(
                        scr[:, :w_], zaggT[:, a:bnd], 0.0, zaggT[:, a:bnd],
                        op0=OP.add, op1=OP.mult,
                        accum_out=sqp[:, j:j + 1])
                nc.vector.tensor_reduce(stats[:, 1:2], sqp[:], axis=AX.X,
                                        op=OP.add)

                stb = dpool.tile([128, 2], F32, tag="stb")
                nc.sync.dma_start(stb[:, :], stats[:])
                nc.gpsimd.collective_compute(
                    "AllReduce", OP.add, replica_groups=rg,
                    ins=[stb[:, :]], outs=[stb[:, :]])
                gstats = npool.tile([128, 2], F32, tag="gstats")
                nc.sync.dma_start(gstats[:], stb[:, :])

                mu = npool.tile([128, 1], F32, tag="mu")
                nc.vector.tensor_scalar_mul(mu[:], gstats[:, 0:1],
                                            1.0 / (NLOC * NCORES))
                msq = npool.tile([128, 1], F32, tag="msq")
                nc.vector.tensor_scalar_mul(msq[:], gstats[:, 1:2],
                                            1.0 / (NLOC * NCORES))
                mu2 = npool.tile([128, 1], F32, tag="mu2")
                nc.vector.tensor_tensor(mu2[:], mu[:], mu[:], OP.mult)
                var = npool.tile([128, 1], F32, tag="var")
                nc.vector.scalar_tensor_tensor(var[:], msq[:], EPS, mu2[:],
                                               op0=OP.add, op1=OP.subtract)
                sd = npool.tile([128, 1], F32, tag="sd")
                nc.scalar.activation(sd[:], var[:], AF.Sqrt,
                                     bias=zeros1[:, 0:1], scale=1.0)
                rstd = npool.tile([128, 1], F32, tag="rstd")
                nc.vector.reciprocal(rstd[:], sd[:])
                nmr = npool.tile([128, 1], F32, tag="nmr")
                nc.vector.scalar_tensor_tensor(nmr[:], mu[:], -1.0, rstd[:],
                                               op0=OP.mult, op1=OP.mult)
                if l < L - 1:
                    zn2 = npool.tile([128, NLOC], F16, tag="znT")
                else:
                    zn2 = hpool.tile([128, NLOC], F32, tag="znTf")
                nc.scalar.activation(zn2[:], zaggT[:, :NLOC], AF.Relu,
                                     bias=nmr[:, 0:1], scale=rstd[:, 0:1])
                znT = zn2

            nc.sync.dma_start(out_t[:, :], znT[:])

    nc.compile()
    return nc


_CACHE = {}


def _get_program(key, *args, **kw):
    if key not in _CACHE:
        _CACHE[key] = _build_program(*args, **kw)
    return _CACHE[key]


def kernel(x, edge_index, W, a_src, a_dst, b):
    x = np.asarray(x, np.float32)
    edge_index = np.asarray(edge_index)
    W = np.asarray(W, np.float32)
    a_src = np.asarray(a_src, np.float32)
    a_dst = np.asarray(a_dst, np.float32)

    hp = _build_host(x, edge_index, W, a_src, a_dst)
    NLOC, G, NPAD, TOTK = hp["NLOC"], hp["G"], hp["NPAD"], hp["TOTK"]
    key = (NLOC, G, tuple(hp["Klo"]), tuple(hp["Khi"]))
    nc = _get_program(key, NLOC, G, NPAD, hp["HALF"], hp["Klo"], hp["Khi"],
                      hp["offs"], TOTK)

    ident = np.eye(128, dtype=np.float16)
    W16 = W.astype(np.float16)
    in_maps = []
    for c in range(NCORES):
        in_maps.append({
            "xt": hp["xt_maps"][c],
            "w": W16,
            "wa": hp["Wa"],
            "idx": hp["idx_maps"][c],
            "ident": ident,
        })

    res = run_bass_kernel_spmd(nc, in_maps, core_ids=list(range(NCORES)))

    N = x.shape[0]
    out = np.empty((N, 128), np.float32)
    for c in range(NCORES):
        zc = res.results[c]["zout"]  # [128, NLOC]
        out[c * NLOC + hp["perms"][c]] = zc.T
    return out


def _install_ntff_hook():
    """Make trace=True work when antenv.axon_hooks is absent (agent image)."""
    import sys as _sys
    import types as _types
    try:
        from antenv.axon_hooks import get_axon_ntff_profile_hook  # noqa: F401
        return
    except ImportError:
        pass
    try:
        import trn_agent_boot.trn_boot as _tb
        hook = _tb._ntff_profile_via_ctypes("/opt/axon/libaxon_pjrt.so")
        mod = _types.ModuleType("antenv.axon_hooks")
        mod.get_axon_ntff_profile_hook = lambda: hook
        mod.set_axon_ntff_profile_hook = lambda h: None
        _sys.modules["antenv.axon_hooks"] = mod
        import concourse.bass_utils as _bu
        _bu.upload_artifacts = lambda tmpdir: tmpdir
    except Exception:
        pass


def profile_exec_ns(inputs):
    """Run once with tracing and return HW exec time in ns (or None)."""
    _install_ntff_hook()
    x = np.asarray(inputs["x"], np.float32)
    hp = _build_host(x, np.asarray(inputs["edge_index"]),
                     np.asarray(inputs["W"], np.float32),
                     np.asarray(inputs["a_src"], np.float32),
                     np.asarray(inputs["a_dst"], np.float32))
    key = (hp["NLOC"], hp["G"], tuple(hp["Klo"]), tuple(hp["Khi"]))
    nc = _get_program(key, hp["NLOC"], hp["G"], hp["NPAD"], hp["HALF"],
                      hp["Klo"], hp["Khi"], hp["offs"], hp["TOTK"])
    ident = np.eye(128, dtype=np.float16)
    in_maps = [{"xt": hp["xt_maps"][c],
                "w": np.asarray(inputs["W"], np.float16),
                "wa": hp["Wa"], "idx": hp["idx_maps"][c], "ident": ident}
               for c in range(NCORES)]
    try:
        res = run_bass_kernel_spmd(nc, in_maps, core_ids=list(range(NCORES)),
                                   trace=True)
        return res.exec_time_ns
    except Exception as ex:
        print("profile failed:", ex)
        return None

